# revision 29
# baseline (speedup 1.0000x reference)
"""Trainium2 Bass kernel for nn_AutoRegressive (dense transformer decoder).

Model: B=4 packed text+audio sequences, L=768, D=1024, 16 heads, DFF=4096,
6 norm-first decoder layers (self-attn w/ prefix-LM mask, cross-attn to the
packed embedding, FFN), weight-tied audio head. fp32 inputs/outputs.

Sharding: DP4 x TP2 over 8 cores. Core pair (2i, 2i+1) owns batch item i;
within a pair the 16 heads split 8+8 and DFF splits 2048+2048. Three
pair-AllReduces per layer (attn-out partials, FFN partials), each split in
two bf16 chunks overlapped with the producing projection.

Layout: activations are feature-major (x^T: [D, L], D on partitions).
Weights are pre-transposed on the host so each [128,128] stationary tile
DMAs straight into SBUF (no PE transposes, no PSUM->SBUF weight copies).
Matmuls use float32r (TF32-like e8m11, full-rate PE path). LayerNorm
partition reductions use ones-vector matmuls col-packed 4-wide; softmax
denominators come from a ones column in the AV stationary operand.
"""
import os
import numpy as np

import concourse.bass as bass
from concourse import bacc
import concourse.mybir as mybir
import concourse.tile as tile
from concourse.bass_utils import run_bass_kernel_spmd
from concourse.masks import make_identity

F32 = mybir.dt.float32
F32R = mybir.dt.float32r
BF16 = mybir.dt.bfloat16
I16 = mybir.dt.int16
AF = mybir.ActivationFunctionType
OP = mybir.AluOpType

B, Tt, Ta, L, D, H, DH, DFF, NL = 4, 128, 640, 768, 1024, 16, 64, 4096, 6
VT, VA = 256, 1026
NLAYERS = int(os.environ.get("KERNEL_NL", str(NL)))
P = 128
NT = L // P          # 6 sequence tiles
DK = D // P          # 8 feature tiles
QO = 4               # q out-tiles (local 512 dims)
QKVO = 12            # qkv out-tiles (local 1536)
F1O = 16             # ffn hidden out-tiles (local 2048)
HEADO = 5            # head out-tiles (640-row padded vocab slab)
NEG = -1.0e30
CH = (0, 384, L)
# self-attn (key-tile, query-chunk) pairs that are fully masked for every
# core: keys >= 128*tt >= 384 can never be text prefix (tl <= 128) and are
# strictly above the causal diagonal of chunk 0 (q <= 383).
SA_SKIP = {(3, 0), (4, 0), (5, 0)}


def _build_nc():
    nc = bacc.Bacc(None)

    comb = nc.declare_dram_parameter("comb", [VT + VA + 1, D], F32, isOutput=False)
    ids16 = nc.declare_dram_parameter("ids16", [P, L // 16], I16, isOutput=False)
    peT_d = nc.declare_dram_parameter("peT", [D, L], F32, isOutput=False)
    mlo_d = nc.declare_dram_parameter("mlo", [P, NT], F32, isOutput=False)
    mhi_d = nc.declare_dram_parameter("mhi", [P, NT], F32, isOutput=False)
    # pre-transposed weights: [o, p, k*128+c] = W_local[o*128+c, k*128+p]
    wqkv_sa = nc.declare_dram_parameter("wqkv_sa", [NLAYERS, QKVO, P, D], F32R, isOutput=False)
    wout_sa = nc.declare_dram_parameter("wout_sa", [NLAYERS, DK, P, 512], F32R, isOutput=False)
    wqkv_ca = nc.declare_dram_parameter("wqkv_ca", [NLAYERS, QKVO, P, D], F32R, isOutput=False)
    wout_ca = nc.declare_dram_parameter("wout_ca", [NLAYERS, DK, P, 512], F32R, isOutput=False)
    w1_d = nc.declare_dram_parameter("w1", [NLAYERS, F1O, P, D], F32R, isOutput=False)
    w2_d = nc.declare_dram_parameter("w2", [NLAYERS, DK, P, 2048], F32R, isOutput=False)
    headw = nc.declare_dram_parameter("headw", [HEADO, P, D], F32R, isOutput=False)
    logits = nc.declare_dram_parameter("logits", [HEADO * P, L], F32, isOutput=True)
    DBG = bool(int(os.environ.get("KERNEL_DEBUG", "0")))
    dbg = {}
    if DBG:
        for nm, shp in [("dx0", [D, L]), ("dmem", [D, L]), ("dh1", [D, L]),
                        ("dqkv", [1536, L]), ("dctx", [512, L]),
                        ("dx1", [D, L]), ("dh2", [D, L]),
                        ("dqkv2", [1536, L]), ("dctx2", [512, L]),
                        ("dx2", [D, L]), ("dff", [2048, L]), ("dx3", [D, L]),
                        ("dvts", [P, 512]), ("dinv", [P, L])]:
            dbg[nm] = nc.declare_dram_parameter(nm, shp, F32R, isOutput=True)

    cc_in = nc.dram_tensor("cc_in", [D, L], BF16)
    cc_out = nc.dram_tensor("cc_out", [D, L], BF16)
    GROUPS = [[0, 1], [2, 3], [4, 5], [6, 7]]

    from contextlib import ExitStack
    with tile.TileContext(nc) as tc, ExitStack() as S:
        state = S.enter_context(tc.tile_pool(name="state", bufs=1))
        wrp = S.enter_context(tc.tile_pool(name="wrp", bufs=2))
        prb = S.enter_context(tc.tile_pool(name="prb", bufs=5))
        evp = S.enter_context(tc.tile_pool(name="evp", bufs=2))

        ident = state.tile([P, P], F32)
        make_identity(nc, ident)
        identR = state.tile([P, P], F32R)
        nc.scalar.copy(identR, ident)
        ones1 = state.tile([P, 1], F32)
        nc.vector.memset(ones1, 1.0)
        onesR = state.tile([P, 1], F32R)
        nc.scalar.copy(onesR, ones1)
        onesrowR = state.tile([P, P], F32R)
        nc.vector.memset(onesrowR.bitcast(F32), 1.0)
        invr = state.tile([P, L], F32R)
        epst = state.tile([1, 1], F32)
        nc.vector.memset(epst, 1e-5)

        xT = state.tile([P, DK, L], F32R)
        memT = state.tile([P, DK, L], F32R)
        hT = state.tile([P, DK, L], F32R)
        big = state.tile([P, F1O, L], F32R)      # qkvT (12 slices) / h1T (16)
        ctxT = state.tile([P, QO, L], F32R)
        maskb = state.tile([P, NT, L], BF16)     # additive mask^T (0 / -1e30)
        mu_s = state.tile([1, L], F32)
        var_s = state.tile([1, L], F32)
        sd_s = state.tile([1, L], F32)
        mub = state.tile([P, L], F32)
        rsb = state.tile([P, L], F32)
        invb = mub  # disjoint lifetimes: mub lives in LN, invb in attn epilogue
        mlo_t = state.tile([P, NT], F32)
        mhi_t = state.tile([P, NT], F32)
        idx_t = state.tile([P, L // 16], I16)
        # persistent AV stationaries: [:, b, 0, 0:64]=v(hh0) col 64=ones;
        # [:, b, 1, 64:128]=v(hh1), col 0=ones (denom row 0)
        vts = state.tile([P, 2, 2, P], F32R)
        nc.vector.memset(vts.bitcast(F32), 0.0)
        for b_ in range(2):
            nc.vector.memset(vts[:, b_, 0, 64:65].bitcast(F32), 1.0)
            nc.vector.memset(vts[:, b_, 1, 0:1].bitcast(F32), 1.0)

        nc.sync.dma_start(out=mlo_t, in_=mlo_d[:, :])
        nc.sync.dma_start(out=mhi_t, in_=mhi_d[:, :])
        nc.sync.dma_start(out=idx_t, in_=ids16[:, :])

        # ---------------- mask build ----------------
        # maskb[k, q] = -1e30 * (relu(lo_k - q) + relu(q + 1 - hi_k))
        iot = mub  # staging before first LN
        nc.gpsimd.iota(iot, pattern=[[1, L]], base=0, channel_multiplier=0,
                       allow_small_or_imprecise_dtypes=True)
        with tc.tile_pool(name="mk", bufs=2) as mkp:
            for tt in range(NT):
                t1 = mkp.tile([P, L], F32, tag="mk")
                nc.scalar.activation(t1, iot, AF.Relu,
                                     bias=mlo_t[:, tt:tt + 1], scale=-1.0)
                t2 = mkp.tile([P, L], F32, tag="mk")
                nc.scalar.activation(t2, iot, AF.Relu,
                                     bias=mhi_t[:, tt:tt + 1], scale=1.0)
                nc.vector.tensor_add(out=t1, in0=t1, in1=t2)
                nc.vector.tensor_scalar_mul(maskb[:, tt, :], t1, NEG)

        # ---------------- embedding ----------------
        with tc.tile_pool(name="emb_ps", bufs=3, space="PSUM") as eps_p, \
             tc.tile_pool(name="emb_sb", bufs=2) as emb_sb:
            for tt in range(NT):
                g = emb_sb.tile([P, 1, D], F32, tag="grow")
                nc.gpsimd.dma_gather(g, comb[:, :],
                                     idx_t[:, 8 * tt:8 * (tt + 1)],
                                     num_idxs=P, num_idxs_reg=P, elem_size=D)
                for f in range(DK):
                    tp = eps_p.tile([P, P], F32, tag="tp")
                    nc.tensor.transpose(tp, g[:, 0, f * P:(f + 1) * P], ident)
                    pe_sl = emb_sb.tile([P, P], F32, tag="pe")
                    nc.sync.dma_start(
                        out=pe_sl,
                        in_=peT_d[f * P:(f + 1) * P, tt * P:(tt + 1) * P])
                    nc.vector.tensor_add(
                        out=xT[:, f, tt * P:(tt + 1) * P],
                        in0=tp, in1=pe_sl)
                    nc.scalar.copy(
                        memT[:, f, tt * P:(tt + 1) * P],
                        xT[:, f, tt * P:(tt + 1) * P].bitcast(F32))

        def dump(nm, tile_ap, n):
            if not DBG:
                return
            d = dbg[nm]
            for o in range(n):
                nc.sync.dma_start(out=d[o * P:(o + 1) * P, :],
                                  in_=tile_ap[:, o, :])

        if DBG:
            dump("dx0", xT, DK)
            dump("dmem", memT, DK)

        # ---------------- helpers ----------------
        def layernorm(src):
            """LN over partition dim of src -> hT (no affine; w=1, b=0)."""
            with tc.tile_pool(name="ln_ps", bufs=1, space="PSUM") as lps:
                s1 = [lps.tile([1, 384], F32, tag=f"s1{c}", name=f"s1{c}") for c in range(2)]
                s2 = [lps.tile([1, 384], F32, tag=f"s2{c}", name=f"s2{c}") for c in range(2)]
                for k in range(DK):
                    st, sp = (k == 0), (k == DK - 1)
                    for c in range(2):
                        sl = slice(CH[c], CH[c + 1])
                        sq = prb.tile([P, 384], F32R, tag="p384")
                        nc.vector.tensor_tensor(
                            out=sq, in0=src[:, k, sl].bitcast(F32),
                            in1=src[:, k, sl].bitcast(F32), op=OP.mult)
                        nc.tensor.matmul(s1[c], onesR, src[:, k, sl],
                                         start=st, stop=sp)
                        nc.tensor.matmul(s2[c], onesR, sq, start=st, stop=sp)
                for c in range(2):
                    sl = slice(CH[c], CH[c + 1])
                    nc.vector.tensor_scalar_mul(mu_s[:, sl], s1[c], 1.0 / D)
                    nc.vector.tensor_mul(out=var_s[:, sl], in0=mu_s[:, sl],
                                         in1=mu_s[:, sl])
                    nc.vector.scalar_tensor_tensor(
                        out=var_s[:, sl], in0=s2[c], scalar=1.0 / D,
                        in1=var_s[:, sl], op0=OP.mult, op1=OP.subtract)
            nc.scalar.activation(sd_s, var_s, AF.Sqrt, bias=epst[0:1, 0:1])
            nc.vector.reciprocal_approx_fast(out=sd_s, in_=sd_s)
            nc.gpsimd.partition_broadcast(mub, mu_s[0:1, :])
            nc.gpsimd.partition_broadcast(rsb, sd_s[0:1, :])
            for k in range(DK):
                nc.vector.tensor_tensor(out=hT[:, k, :],
                                        in0=src[:, k, :].bitcast(F32),
                                        in1=mub, op=OP.subtract)
                nc.vector.tensor_mul(out=hT[:, k, :],
                                     in0=hT[:, k, :].bitcast(F32), in1=rsb)

        def proj(w_ap, n_o, kcnt, rhs_fn, out_fn, pool=None, tag="acc"):
            """acc[o] = sum_k W^T[o,k] @ rhs(o,k); out_fn(o, acc_psum).

            w_ap: DRAM AP [n_o, 128, kcnt*128], pre-transposed tiles.
            """
            from contextlib import nullcontext
            cm = (nullcontext(pool) if pool is not None
                  else tc.tile_pool(name="pj_ps", bufs=2, space="PSUM"))
            with cm as pps:
                for o in range(n_o):
                    wslab = wrp.tile([P, F1O * P], F32R, tag="wslab")
                    half = kcnt * P // 2
                    nc.sync.dma_start(out=wslab[:, 0:half],
                                      in_=w_ap[o, :, 0:half])
                    nc.sync.dma_start(out=wslab[:, half:kcnt * P],
                                      in_=w_ap[o, :, half:kcnt * P])
                    acc = pps.tile([P, L], F32, tag=tag)
                    for k in range(kcnt):
                        wT = wslab[:, k * P:(k + 1) * P]
                        rhs = rhs_fn(o, k)
                        st, sp = (k == 0), (k == kcnt - 1)
                        nc.tensor.matmul(acc[:, 0:512], wT, rhs[:, 0:512],
                                         start=st, stop=sp)
                        nc.tensor.matmul(acc[:, 512:L], wT, rhs[:, 512:L],
                                         start=st, stop=sp)
                    out_fn(o, acc)

        def attention(masked):
            """big[:, 0:12] = qkvT (2 heads per 128-tile) -> ctxT."""
            skip = SA_SKIP if masked else set()
            cmax = {c: max(tt for tt in range(NT) if (tt, c) not in skip)
                    for c in range(2)}
            cmin = {c: min(tt for tt in range(NT) if (tt, c) not in skip)
                    for c in range(2)}
            with tc.tile_pool(name="at_sps", bufs=2, space="PSUM") as sps, \
                 tc.tile_pool(name="at_cps", bufs=4, space="PSUM") as cps, \
                 tc.tile_pool(name="at_vps", bufs=1, space="PSUM") as vps, \
                 tc.tile_pool(name="at_fps", bufs=1, space="PSUM") as fps:
                # HAM warmers: dependency-free matmuls into a dead bank,
                # emitted where the in-order PE stream waits on ScalarE
                # (exp) or the epilogue chain, so the clock gate stays open.
                fillt = fps.tile([P, 512], F32, tag="fill")

                def filler(n):
                    for _ in range(n):
                        nc.tensor.matmul(fillt, identR, big[:, 0, 0:512],
                                         start=True, stop=True)
                for j in range(QO):          # head pair j: heads 2j, 2j+1
                    ctx = [[cps.tile([P, 384], F32, tag="ctx",
                                          name=f"ctx{hh}{c}")
                            for c in range(2)] for hh in range(2)]
                    for tt in range(NT):
                        prob = [[None, None], [None, None]]
                        for hh in range(2):
                            hb = 64 * hh
                            kT = big[hb:hb + 64, 4 + j, tt * P:(tt + 1) * P]
                            for c in range(2):
                                if (tt, c) in skip:
                                    continue
                                sc = sps.tile([P, 384], F32, tag="sc")
                                qT = big[hb:hb + 64, j, CH[c]:CH[c + 1]]
                                nc.tensor.matmul(sc, kT, qT,
                                                 start=True, stop=True)
                                if masked:
                                    nc.vector.scalar_tensor_tensor(
                                        out=sc,
                                        in0=maskb[:, tt, CH[c]:CH[c + 1]],
                                        scalar=1.0, in1=sc,
                                        op0=OP.mult, op1=OP.add)
                                pr = prb.tile([P, 384], F32R, tag="p384")
                                nc.scalar.activation(pr, sc, AF.Exp,
                                                     scale=0.125)
                                prob[hh][c] = pr
                        filler(2)
                        for hh in range(2):
                            hb = 64 * hh
                            # v_tok: transpose vT slice [64, 128] -> [128, 64]
                            vtp = vps.tile([P, 64], F32R, tag="vtp")
                            nc.tensor.transpose(
                                vtp,
                                big[hb:hb + 64, 8 + j, tt * P:(tt + 1) * P],
                                identR[hb:hb + 64, hb:hb + 64])
                            vsl = slice(0, 64) if hh == 0 else slice(64, 128)
                            nc.scalar.copy(vts[:, tt % 2, hh, vsl],
                                           vtp.bitcast(F32))
                            lhs = (vts[:, tt % 2, 0, 0:65] if hh == 0
                                   else vts[:, tt % 2, 1, 0:128])
                            m_sl = slice(0, 65) if hh == 0 else slice(0, 128)
                            for c in range(2):
                                if (tt, c) in skip:
                                    continue
                                nc.tensor.matmul(
                                    ctx[hh][c][m_sl, :], lhs, prob[hh][c],
                                    start=(tt == cmin[c]),
                                    stop=(tt == cmax[c]))
                        filler(1)
                    filler(4)
                    # epilogue: first evacuate each ctx PSUM tile to SBUF
                    # with one DVE copy so the pool frees for the next head
                    # pair's AV immediately; then normalize from SBUF. The
                    # approx reciprocal only works at partition base 0, so
                    # hh0's denominator (row 64) moves there via a 1-column
                    # PE matmul first.
                    # hT is dead during attention (consumed by the QKV
                    # projection, rewritten by the next LN) - use its slices
                    # as the evacuation target.
                    cs = [[None, None], [None, None]]
                    for hh in range(2):
                        for c in range(2):
                            t = hT[:, (0 if hh == 0 else 4) + j,
                                   CH[c]:CH[c] + 384]
                            nc.vector.tensor_copy(out=t, in_=ctx[hh][c])
                            cs[hh][c] = t
                    for hh in range(2):
                        hb = 64 * hh
                        dr = 64 if hh == 0 else 0
                        for c in range(2):
                            sl = slice(CH[c], CH[c + 1])
                            if hh == 0:
                                db = sps.tile([P, 384], F32, tag="sc",
                                              name=f"db{c}")
                                nc.tensor.matmul(db[0:1, :],
                                                 onesrowR[dr:dr + 1, 0:1],
                                                 cs[0][c][dr:dr + 1, :],
                                                 start=True, stop=True)
                                den0 = db[0:1, :]
                            else:
                                den0 = cs[1][c][0:1, :].bitcast(F32)
                            t1 = prb.tile([P, 384], F32R, tag="p384")
                            nc.vector.reciprocal_approx_fast(
                                out=t1[0:1, :].bitcast(F32), in_=den0)
                            t1r = prb.tile([P, 384], F32R, tag="p384")
                            nc.scalar.copy(t1r[0:1, :],
                                           t1[0:1, :].bitcast(F32))
                            ib = sps.tile([P, 384], F32, tag="sc",
                                          name=f"ib{hh}{c}")
                            nc.tensor.matmul(ib, onesrowR[0:1, :],
                                             t1r[0:1, :],
                                             start=True, stop=True)
                            t2 = prb.tile([P, 384], F32R, tag="p384")
                            nc.scalar.copy(t2[hb:hb + 64, :],
                                           ib[hb:hb + 64, :])
                            nc.vector.tensor_mul(
                                out=ctxT[hb:hb + 64, j, sl],
                                in0=cs[hh][c][hb:hb + 64, :].bitcast(F32),
                                in1=t2[hb:hb + 64, :].bitcast(F32))

        def make_out_evac():
            """Evacuate out-proj partials as bf16, AllReduce in two chunks
            overlapped with the second half of the projection, and add the
            reduced result back into xT."""
            def chunk(o_lo, o_hi):
                nc.gpsimd.collective_compute(
                    "AllReduce", OP.add, replica_groups=GROUPS,
                    ins=[cc_in[o_lo * P:o_hi * P, :]],
                    outs=[cc_out[o_lo * P:o_hi * P, :]])
                for o in range(o_lo, o_hi):
                    rr = evp.tile([P, L], BF16, tag="rrb")
                    nc.sync.dma_start(out=rr, in_=cc_out[o * P:(o + 1) * P, :])
                    nc.vector.tensor_tensor(out=xT[:, o, :],
                                            in0=xT[:, o, :].bitcast(F32),
                                            in1=rr, op=OP.add)

            def evac(o, acc):
                ev = evp.tile([P, L], BF16, tag="evb")
                nc.vector.tensor_copy(out=ev, in_=acc)
                nc.sync.dma_start(out=cc_in[o * P:(o + 1) * P, :], in_=ev)
                if o == 2:
                    chunk(0, 3)
                elif o == 5:
                    chunk(3, 6)
                elif o == 7:
                    chunk(6, 8)
            return evac

        def qkv_evac(o, acc):
            nc.vector.tensor_copy(out=big[:, o, :], in_=acc)

        def relu_evac(o, acc):
            nc.scalar.activation(big[:, o, :], acc, AF.Relu)

        # ---------------- layers ----------------
        for l in range(NLAYERS):
            # ---- self-attention ----
            layernorm(xT)
            if l == 0:
                dump("dh1", hT, DK)
            proj(wqkv_sa[l], QKVO, DK, lambda o, k: hT[:, k, :], qkv_evac)
            if l == 0:
                dump("dqkv", big, QKVO)
            attention(masked=True)
            if l == 0:
                dump("dctx", ctxT, QO)
            # CA k/v depend only on the static memT: compute them during the
            # SA out-projection + AllReduce window to keep the PE busy.
            with tc.tile_pool(name="ov_ps", bufs=2, space="PSUM") as ovp:
                proj(wqkv_ca[l][4:QKVO], QKVO - 4, DK,
                     lambda o, k: memT[:, k, :],
                     lambda o, acc: qkv_evac(o + 4, acc),
                     pool=ovp, tag="kv")
                proj(wout_sa[l], DK, QO, lambda o, k: ctxT[:, k, :],
                     make_out_evac(), pool=ovp, tag="out")
            if l == 0:
                dump("dx1", xT, DK)

            # ---- cross-attention (k/v from packed embedding memT) ----
            layernorm(xT)
            if l == 0:
                dump("dh2", hT, DK)
            proj(wqkv_ca[l][0:4], 4, DK,
                 lambda o, k: hT[:, k, :], qkv_evac)
            if l == 0:
                dump("dqkv2", big, QKVO)
            attention(masked=False)
            if l == 0:
                dump("dctx2", ctxT, QO)
            proj(wout_ca[l], DK, QO, lambda o, k: ctxT[:, k, :],
                 make_out_evac())
            if l == 0:
                dump("dx2", xT, DK)

            # ---- FFN ----
            layernorm(xT)
            proj(w1_d[l], F1O, DK, lambda o, k: hT[:, k, :], relu_evac)
            if l == 0:
                dump("dff", big, F1O)
            proj(w2_d[l], DK, 2 * DK, lambda o, k: big[:, k, :],
                 make_out_evac())
            if l == 0:
                dump("dx3", xT, DK)

        # ---------------- head (vocab split across the pair) ----------------
        def head_evac(o, acc):
            for c in range(2):
                ev = prb.tile([P, 384], F32R, tag="p384")
                nc.scalar.copy(ev.bitcast(F32), acc[:, CH[c]:CH[c + 1]])
                nc.sync.dma_start(
                    out=logits[o * P:(o + 1) * P, CH[c]:CH[c + 1]],
                    in_=ev.bitcast(F32))

        proj(headw, HEADO, DK, lambda o, k: xT[:, k, :], head_evac)
        if DBG:
            nc.sync.dma_start(out=dbg["dvts"][:, :],
                              in_=vts[:, :, :, :].bitcast(F32R))
            nc.sync.dma_start(out=dbg["dinv"][:, :], in_=invb.bitcast(F32R))

    nc.finalize()
    return nc


# ---------------------------------------------------------------------------
# host side
# ---------------------------------------------------------------------------

def _pe_table(length, d):
    pos = np.arange(length, dtype=np.float32)[:, None]
    div = np.exp(np.arange(0, d, 2, dtype=np.float32) * (-np.log(10000.0) / d))
    ang = pos * div
    out = np.zeros((length, d), np.float32)
    out[:, 0::2] = np.sin(ang)
    out[:, 1::2] = np.cos(ang)
    return out


def _tp(w):
    """[..., O*128, K*128] -> [..., O, 128, K*128] pre-transposed tiles:
    out[..., o, p, k*128+c] = w[..., o*128+c, k*128+p]."""
    lead = w.shape[:-2]
    O, K = w.shape[-2] // P, w.shape[-1] // P
    w = w.reshape(*lead, O, P, K, P)
    axes = tuple(range(len(lead))) + tuple(
        len(lead) + a for a in (0, 3, 2, 1))
    return np.ascontiguousarray(
        w.transpose(*axes).reshape(*lead, O, P, K * P))


_NC_CACHE = {}
LAST_RESULT = {}


def kernel(**inputs):
    f32 = lambda a: np.ascontiguousarray(np.asarray(a, dtype=np.float32))
    text = np.asarray(inputs["text"]).astype(np.int64)
    audio = np.asarray(inputs["audio"]).astype(np.int64)
    tl = np.asarray(inputs["text_len_batch"]).astype(np.int64)
    al = np.asarray(inputs["audio_len_batch"]).astype(np.int64)
    text_table = f32(inputs["text_table"])
    audio_table = f32(inputs["audio_table"])
    sa_in_w = f32(inputs["sa_in_w"])
    sa_out_w = f32(inputs["sa_out_w"])
    ca_in_w = f32(inputs["ca_in_w"])
    ca_out_w = f32(inputs["ca_out_w"])
    ffn_w1 = f32(inputs["ffn_w1"])
    ffn_w2 = f32(inputs["ffn_w2"])

    comb = np.ascontiguousarray(np.concatenate(
        [text_table, audio_table, np.zeros((1, D), np.float32)], axis=0))
    pe_t = _pe_table(Tt, D)
    pe_a = _pe_table(Ta, D)

    in_maps = []
    for c in range(8):
        p, r = c // 2, c % 2
        tlb, alb = int(tl[p]), int(al[p])
        il = tlb + alb

        ids = np.full((L,), VT + VA, dtype=np.int64)  # default: zero row
        ids[:tlb] = text[p, :tlb]
        ids[tlb:il] = VT + audio[p, :alb]
        ids16 = np.ascontiguousarray(np.tile(ids.astype(np.int16).reshape(L // 16, 16).T, (8, 1)))

        pe_pack = np.zeros((L, D), np.float32)
        pe_pack[:tlb] = pe_t[:tlb]
        pe_pack[tlb:il] = pe_a[:alb]
        peT = np.ascontiguousarray(pe_pack.T)

        kk = np.arange(L)
        lo = np.where(kk < tlb, 0, kk).astype(np.float32)
        hi = np.where(kk < tlb, L, il).astype(np.float32)
        mlo = np.ascontiguousarray(lo.reshape(NT, P).T)          # [128, 6]
        mhi = np.ascontiguousarray((1.0 - hi).reshape(NT, P).T)

        sl = slice(512 * r, 512 * (r + 1))

        def qkv_shard(w3):
            qq = w3[:, 0:1024, :][:, sl]
            kx = w3[:, 1024:2048, :][:, sl]
            vv = w3[:, 2048:3072, :][:, sl]
            return np.ascontiguousarray(np.concatenate([qq, kx, vv], axis=1))

        hw = np.zeros((HEADO * P, D), np.float32)
        hw[0:513] = audio_table[513 * r:513 * (r + 1)]

        in_maps.append({
            "comb": comb, "ids16": ids16, "peT": peT,
            "mlo": mlo, "mhi": mhi,
            "wqkv_sa": _tp(qkv_shard(sa_in_w[:NLAYERS])),
            "wout_sa": _tp(np.ascontiguousarray(sa_out_w[:NLAYERS, :, sl])),
            "wqkv_ca": _tp(qkv_shard(ca_in_w[:NLAYERS])),
            "wout_ca": _tp(np.ascontiguousarray(ca_out_w[:NLAYERS, :, sl])),
            "w1": _tp(np.ascontiguousarray(
                ffn_w1[:NLAYERS, 2048 * r:2048 * (r + 1), :])),
            "w2": _tp(np.ascontiguousarray(
                ffn_w2[:NLAYERS, :, 2048 * r:2048 * (r + 1)])),
            "headw": _tp(hw),
        })

    if "nc" not in _NC_CACHE:
        _NC_CACHE["nc"] = _build_nc()
    nc = _NC_CACHE["nc"]
    trace = bool(int(os.environ.get("KERNEL_TRACE", "0")))
    r = run_bass_kernel_spmd(nc, in_maps, core_ids=list(range(8)), trace=trace)
    LAST_RESULT["r"] = r
    res = r.results

    out = np.empty((B, L, VA), np.float32)
    for p in range(B):
        ev = res[2 * p]["logits"]
        od = res[2 * p + 1]["logits"]
        out[p] = np.concatenate([ev[0:513], od[0:513]], axis=0).T
    return out


# revision 30
# speedup vs baseline: 1.0159x; 1.0159x over previous
"""Trainium2 Bass kernel for nn_AutoRegressive (dense transformer decoder).

Model: B=4 packed text+audio sequences, L=768, D=1024, 16 heads, DFF=4096,
6 norm-first decoder layers (self-attn w/ prefix-LM mask, cross-attn to the
packed embedding, FFN), weight-tied audio head. fp32 inputs/outputs.

Sharding: DP4 x TP2 over 8 cores. Core pair (2i, 2i+1) owns batch item i;
within a pair the 16 heads split 8+8 and DFF splits 2048+2048. Three
pair-AllReduces per layer (attn-out partials, FFN partials), each split in
two bf16 chunks overlapped with the producing projection.

Layout: activations are feature-major (x^T: [D, L], D on partitions).
Weights are pre-transposed on the host so each [128,128] stationary tile
DMAs straight into SBUF (no PE transposes, no PSUM->SBUF weight copies).
Matmuls use float32r (TF32-like e8m11, full-rate PE path). LayerNorm
partition reductions use ones-vector matmuls col-packed 4-wide; softmax
denominators come from a ones column in the AV stationary operand.
"""
import os
import numpy as np

import concourse.bass as bass
from concourse import bacc
import concourse.mybir as mybir
import concourse.tile as tile
from concourse.bass_utils import run_bass_kernel_spmd
from concourse.masks import make_identity

F32 = mybir.dt.float32
F32R = mybir.dt.float32r
BF16 = mybir.dt.bfloat16
I16 = mybir.dt.int16
AF = mybir.ActivationFunctionType
OP = mybir.AluOpType

B, Tt, Ta, L, D, H, DH, DFF, NL = 4, 128, 640, 768, 1024, 16, 64, 4096, 6
VT, VA = 256, 1026
NLAYERS = int(os.environ.get("KERNEL_NL", str(NL)))
P = 128
NT = L // P          # 6 sequence tiles
DK = D // P          # 8 feature tiles
QO = 4               # q out-tiles (local 512 dims)
QKVO = 12            # qkv out-tiles (local 1536)
F1O = 16             # ffn hidden out-tiles (local 2048)
HEADO = 5            # head out-tiles (640-row padded vocab slab)
NEG = -1.0e30
CH = (0, 384, L)
# self-attn (key-tile, query-chunk) pairs that are fully masked for every
# core: keys >= 128*tt >= 384 can never be text prefix (tl <= 128) and are
# strictly above the causal diagonal of chunk 0 (q <= 383).
SA_SKIP = {(3, 0), (4, 0), (5, 0)}


def _build_nc():
    nc = bacc.Bacc(None)

    comb = nc.declare_dram_parameter("comb", [VT + VA + 1, D], F32, isOutput=False)
    ids16 = nc.declare_dram_parameter("ids16", [P, L // 16], I16, isOutput=False)
    peT_d = nc.declare_dram_parameter("peT", [D, L], F32, isOutput=False)
    mlo_d = nc.declare_dram_parameter("mlo", [P, NT], F32, isOutput=False)
    mhi_d = nc.declare_dram_parameter("mhi", [P, NT], F32, isOutput=False)
    # pre-transposed weights: [o, p, k*128+c] = W_local[o*128+c, k*128+p]
    wqkv_sa = nc.declare_dram_parameter("wqkv_sa", [NLAYERS, QKVO, P, D], F32R, isOutput=False)
    wout_sa = nc.declare_dram_parameter("wout_sa", [NLAYERS, DK, P, 512], F32R, isOutput=False)
    wqkv_ca = nc.declare_dram_parameter("wqkv_ca", [NLAYERS, QKVO, P, D], F32R, isOutput=False)
    wout_ca = nc.declare_dram_parameter("wout_ca", [NLAYERS, DK, P, 512], F32R, isOutput=False)
    w1_d = nc.declare_dram_parameter("w1", [NLAYERS, F1O, P, D], F32R, isOutput=False)
    w2_d = nc.declare_dram_parameter("w2", [NLAYERS, DK, P, 2048], F32R, isOutput=False)
    headw = nc.declare_dram_parameter("headw", [HEADO, P, D], F32R, isOutput=False)
    logits = nc.declare_dram_parameter("logits", [HEADO * P, L], F32, isOutput=True)
    DBG = bool(int(os.environ.get("KERNEL_DEBUG", "0")))
    dbg = {}
    if DBG:
        for nm, shp in [("dx0", [D, L]), ("dmem", [D, L]), ("dh1", [D, L]),
                        ("dqkv", [1536, L]), ("dctx", [512, L]),
                        ("dx1", [D, L]), ("dh2", [D, L]),
                        ("dqkv2", [1536, L]), ("dctx2", [512, L]),
                        ("dx2", [D, L]), ("dff", [2048, L]), ("dx3", [D, L]),
                        ("dvts", [P, 512]), ("dinv", [P, L])]:
            dbg[nm] = nc.declare_dram_parameter(nm, shp, F32R, isOutput=True)

    cc_in = nc.dram_tensor("cc_in", [D, L], BF16)
    cc_out = nc.dram_tensor("cc_out", [D, L], BF16)
    GROUPS = [[0, 1], [2, 3], [4, 5], [6, 7]]

    from contextlib import ExitStack
    with tile.TileContext(nc) as tc, ExitStack() as S:
        state = S.enter_context(tc.tile_pool(name="state", bufs=1))
        wrp = S.enter_context(tc.tile_pool(name="wrp", bufs=2))
        prb = S.enter_context(tc.tile_pool(name="prb", bufs=5))
        evp = S.enter_context(tc.tile_pool(name="evp", bufs=2))

        ident = state.tile([P, P], F32)
        make_identity(nc, ident)
        identR = state.tile([P, P], F32R)
        nc.scalar.copy(identR, ident)
        ones1 = state.tile([P, 1], F32)
        nc.vector.memset(ones1, 1.0)
        onesR = state.tile([P, 1], F32R)
        nc.scalar.copy(onesR, ones1)
        onesrowR = state.tile([P, P], F32R)
        nc.vector.memset(onesrowR.bitcast(F32), 1.0)
        invr = state.tile([P, L], F32R)
        epst = state.tile([1, 1], F32)
        nc.vector.memset(epst, 1e-5)

        xT = state.tile([P, DK, L], F32R)
        memT = state.tile([P, DK, L], F32R)
        hT = state.tile([P, DK, L], F32R)
        big = state.tile([P, F1O, L], F32R)      # qkvT (12 slices) / h1T (16)
        ctxT = state.tile([P, QO, L], F32R)
        maskb = state.tile([P, NT, L], BF16)     # additive mask^T (0 / -1e30)
        mu_s = state.tile([1, L], F32)
        var_s = state.tile([1, L], F32)
        sd_s = state.tile([1, L], F32)
        mub = state.tile([P, L], F32)
        rsb = state.tile([P, L], F32)
        invb = mub  # disjoint lifetimes: mub lives in LN, invb in attn epilogue
        mlo_t = state.tile([P, NT], F32)
        mhi_t = state.tile([P, NT], F32)
        idx_t = state.tile([P, L // 16], I16)
        # persistent AV stationaries: [:, b, 0, 0:64]=v(hh0) col 64=ones;
        # [:, b, 1, 64:128]=v(hh1), col 0=ones (denom row 0)
        vts = state.tile([P, 2, 2, P], F32R)
        nc.vector.memset(vts.bitcast(F32), 0.0)
        for b_ in range(2):
            nc.vector.memset(vts[:, b_, 0, 64:65].bitcast(F32), 1.0)
            nc.vector.memset(vts[:, b_, 1, 0:1].bitcast(F32), 1.0)

        nc.sync.dma_start(out=mlo_t, in_=mlo_d[:, :])
        nc.sync.dma_start(out=mhi_t, in_=mhi_d[:, :])
        nc.sync.dma_start(out=idx_t, in_=ids16[:, :])

        # ---------------- mask build ----------------
        # maskb[k, q] = -1e30 * (relu(lo_k - q) + relu(q + 1 - hi_k))
        iot = mub  # staging before first LN
        nc.gpsimd.iota(iot, pattern=[[1, L]], base=0, channel_multiplier=0,
                       allow_small_or_imprecise_dtypes=True)
        with tc.tile_pool(name="mk", bufs=2) as mkp:
            for tt in range(NT):
                t1 = mkp.tile([P, L], F32, tag="mk")
                nc.scalar.activation(t1, iot, AF.Relu,
                                     bias=mlo_t[:, tt:tt + 1], scale=-1.0)
                t2 = mkp.tile([P, L], F32, tag="mk")
                nc.scalar.activation(t2, iot, AF.Relu,
                                     bias=mhi_t[:, tt:tt + 1], scale=1.0)
                nc.vector.tensor_add(out=t1, in0=t1, in1=t2)
                nc.vector.tensor_scalar_mul(maskb[:, tt, :], t1, NEG)

        # ---------------- embedding ----------------
        with tc.tile_pool(name="emb_ps", bufs=3, space="PSUM") as eps_p, \
             tc.tile_pool(name="emb_sb", bufs=2) as emb_sb:
            for tt in range(NT):
                g = emb_sb.tile([P, 1, D], F32, tag="grow")
                nc.gpsimd.dma_gather(g, comb[:, :],
                                     idx_t[:, 8 * tt:8 * (tt + 1)],
                                     num_idxs=P, num_idxs_reg=P, elem_size=D)
                for f in range(DK):
                    tp = eps_p.tile([P, P], F32, tag="tp")
                    nc.tensor.transpose(tp, g[:, 0, f * P:(f + 1) * P], ident)
                    pe_sl = emb_sb.tile([P, P], F32, tag="pe")
                    nc.sync.dma_start(
                        out=pe_sl,
                        in_=peT_d[f * P:(f + 1) * P, tt * P:(tt + 1) * P])
                    nc.vector.tensor_add(
                        out=xT[:, f, tt * P:(tt + 1) * P],
                        in0=tp, in1=pe_sl)
                    nc.scalar.copy(
                        memT[:, f, tt * P:(tt + 1) * P],
                        xT[:, f, tt * P:(tt + 1) * P].bitcast(F32))

        def dump(nm, tile_ap, n):
            if not DBG:
                return
            d = dbg[nm]
            for o in range(n):
                nc.sync.dma_start(out=d[o * P:(o + 1) * P, :],
                                  in_=tile_ap[:, o, :])

        if DBG:
            dump("dx0", xT, DK)
            dump("dmem", memT, DK)

        # ---------------- helpers ----------------
        def layernorm(src):
            """LN over partition dim of src -> hT (no affine; w=1, b=0)."""
            with tc.tile_pool(name="ln_ps", bufs=1, space="PSUM") as lps:
                s1 = [lps.tile([1, 384], F32, tag=f"s1{c}", name=f"s1{c}") for c in range(2)]
                s2 = [lps.tile([1, 384], F32, tag=f"s2{c}", name=f"s2{c}") for c in range(2)]
                for k in range(DK):
                    st, sp = (k == 0), (k == DK - 1)
                    for c in range(2):
                        sl = slice(CH[c], CH[c + 1])
                        sq = prb.tile([P, 384], F32R, tag="p384")
                        nc.vector.tensor_tensor(
                            out=sq, in0=src[:, k, sl].bitcast(F32),
                            in1=src[:, k, sl].bitcast(F32), op=OP.mult)
                        nc.tensor.matmul(s1[c], onesR, src[:, k, sl],
                                         start=st, stop=sp)
                        nc.tensor.matmul(s2[c], onesR, sq, start=st, stop=sp)
                for c in range(2):
                    sl = slice(CH[c], CH[c + 1])
                    nc.vector.tensor_scalar_mul(mu_s[:, sl], s1[c], 1.0 / D)
                    nc.vector.tensor_mul(out=var_s[:, sl], in0=mu_s[:, sl],
                                         in1=mu_s[:, sl])
                    nc.vector.scalar_tensor_tensor(
                        out=var_s[:, sl], in0=s2[c], scalar=1.0 / D,
                        in1=var_s[:, sl], op0=OP.mult, op1=OP.subtract)
            nc.scalar.activation(sd_s, var_s, AF.Sqrt, bias=epst[0:1, 0:1])
            nc.vector.reciprocal_approx_fast(out=sd_s, in_=sd_s)
            nc.gpsimd.partition_broadcast(mub, mu_s[0:1, :])
            nc.gpsimd.partition_broadcast(rsb, sd_s[0:1, :])
            for k in range(DK):
                nc.vector.tensor_tensor(out=hT[:, k, :],
                                        in0=src[:, k, :].bitcast(F32),
                                        in1=mub, op=OP.subtract)
                nc.vector.tensor_mul(out=hT[:, k, :],
                                     in0=hT[:, k, :].bitcast(F32), in1=rsb)

        def proj(w_ap, n_o, kcnt, rhs_fn, out_fn, pool=None, tag="acc"):
            """acc[o] = sum_k W^T[o,k] @ rhs(o,k); out_fn(o, acc_psum).

            w_ap: DRAM AP [n_o, 128, kcnt*128], pre-transposed tiles.
            """
            from contextlib import nullcontext
            cm = (nullcontext(pool) if pool is not None
                  else tc.tile_pool(name="pj_ps", bufs=2, space="PSUM"))
            with cm as pps:
                for o in range(n_o):
                    wslab = wrp.tile([P, F1O * P], F32R, tag="wslab")
                    half = kcnt * P // 2
                    nc.sync.dma_start(out=wslab[:, 0:half],
                                      in_=w_ap[o, :, 0:half])
                    nc.sync.dma_start(out=wslab[:, half:kcnt * P],
                                      in_=w_ap[o, :, half:kcnt * P])
                    acc = pps.tile([P, L], F32, tag=tag)
                    for k in range(kcnt):
                        wT = wslab[:, k * P:(k + 1) * P]
                        rhs = rhs_fn(o, k)
                        st, sp = (k == 0), (k == kcnt - 1)
                        nc.tensor.matmul(acc[:, 0:512], wT, rhs[:, 0:512],
                                         start=st, stop=sp)
                        nc.tensor.matmul(acc[:, 512:L], wT, rhs[:, 512:L],
                                         start=st, stop=sp)
                    out_fn(o, acc)

        def attention(masked):
            """big[:, 0:12] = qkvT (2 heads per 128-tile) -> ctxT."""
            skip = SA_SKIP if masked else set()
            cmax = {c: max(tt for tt in range(NT) if (tt, c) not in skip)
                    for c in range(2)}
            cmin = {c: min(tt for tt in range(NT) if (tt, c) not in skip)
                    for c in range(2)}
            with tc.tile_pool(name="at_sps", bufs=2, space="PSUM") as sps, \
                 tc.tile_pool(name="at_cps", bufs=4, space="PSUM") as cps, \
                 tc.tile_pool(name="at_vps", bufs=2, space="PSUM") as vps:
                for j in range(QO):          # head pair j: heads 2j, 2j+1
                    ctx = [[cps.tile([P, 384], F32, tag="ctx",
                                          name=f"ctx{hh}{c}")
                            for c in range(2)] for hh in range(2)]
                    for tt in range(NT):
                        prob = [[None, None], [None, None]]
                        for hh in range(2):
                            hb = 64 * hh
                            kT = big[hb:hb + 64, 4 + j, tt * P:(tt + 1) * P]
                            for c in range(2):
                                if (tt, c) in skip:
                                    continue
                                sc = sps.tile([P, 384], F32, tag="sc")
                                qT = big[hb:hb + 64, j, CH[c]:CH[c + 1]]
                                nc.tensor.matmul(sc, kT, qT,
                                                 start=True, stop=True)
                                if masked:
                                    nc.vector.scalar_tensor_tensor(
                                        out=sc,
                                        in0=maskb[:, tt, CH[c]:CH[c + 1]],
                                        scalar=1.0, in1=sc,
                                        op0=OP.mult, op1=OP.add)
                                pr = prb.tile([P, 384], F32R, tag="p384")
                                nc.scalar.activation(pr, sc, AF.Exp,
                                                     scale=0.125)
                                prob[hh][c] = pr
                        for hh in range(2):
                            hb = 64 * hh
                            # v_tok: transpose vT slice [64, 128] -> [128, 64]
                            vtp = vps.tile([P, 64], F32R, tag="vtp")
                            nc.tensor.transpose(
                                vtp,
                                big[hb:hb + 64, 8 + j, tt * P:(tt + 1) * P],
                                identR[hb:hb + 64, hb:hb + 64])
                            vsl = slice(0, 64) if hh == 0 else slice(64, 128)
                            nc.scalar.copy(vts[:, tt % 2, hh, vsl],
                                           vtp.bitcast(F32))
                            lhs = (vts[:, tt % 2, 0, 0:65] if hh == 0
                                   else vts[:, tt % 2, 1, 0:128])
                            m_sl = slice(0, 65) if hh == 0 else slice(0, 128)
                            for c in range(2):
                                if (tt, c) in skip:
                                    continue
                                nc.tensor.matmul(
                                    ctx[hh][c][m_sl, :], lhs, prob[hh][c],
                                    start=(tt == cmin[c]),
                                    stop=(tt == cmax[c]))
                    # epilogue: first evacuate each ctx PSUM tile to SBUF
                    # with one DVE copy so the pool frees for the next head
                    # pair's AV immediately; then normalize from SBUF. The
                    # approx reciprocal only works at partition base 0, so
                    # hh0's denominator (row 64) moves there via a 1-column
                    # PE matmul first.
                    # hT is dead during attention (consumed by the QKV
                    # projection, rewritten by the next LN) - use its slices
                    # as the evacuation target.
                    cs = [[None, None], [None, None]]
                    for hh in range(2):
                        for c in range(2):
                            t = hT[:, (0 if hh == 0 else 4) + j,
                                   CH[c]:CH[c] + 384]
                            nc.vector.tensor_copy(out=t, in_=ctx[hh][c])
                            cs[hh][c] = t
                    for hh in range(2):
                        hb = 64 * hh
                        dr = 64 if hh == 0 else 0
                        for c in range(2):
                            sl = slice(CH[c], CH[c + 1])
                            if hh == 0:
                                db = sps.tile([P, 384], F32, tag="sc",
                                              name=f"db{c}")
                                nc.tensor.matmul(db[0:1, :],
                                                 onesrowR[dr:dr + 1, 0:1],
                                                 cs[0][c][dr:dr + 1, :],
                                                 start=True, stop=True)
                                den0 = db[0:1, :]
                            else:
                                den0 = cs[1][c][0:1, :].bitcast(F32)
                            t1 = prb.tile([P, 384], F32R, tag="p384")
                            nc.vector.reciprocal_approx_fast(
                                out=t1[0:1, :].bitcast(F32), in_=den0)
                            t1r = prb.tile([P, 384], F32R, tag="p384")
                            nc.scalar.copy(t1r[0:1, :],
                                           t1[0:1, :].bitcast(F32))
                            ib = sps.tile([P, 384], F32, tag="sc",
                                          name=f"ib{hh}{c}")
                            nc.tensor.matmul(ib, onesrowR[0:1, :],
                                             t1r[0:1, :],
                                             start=True, stop=True)
                            t2 = prb.tile([P, 384], F32R, tag="p384")
                            nc.scalar.copy(t2[hb:hb + 64, :],
                                           ib[hb:hb + 64, :])
                            nc.vector.tensor_mul(
                                out=ctxT[hb:hb + 64, j, sl],
                                in0=cs[hh][c][hb:hb + 64, :].bitcast(F32),
                                in1=t2[hb:hb + 64, :].bitcast(F32))

        def make_out_evac():
            """Evacuate out-proj partials as bf16, AllReduce in two chunks
            overlapped with the second half of the projection, and add the
            reduced result back into xT."""
            def chunk(o_lo, o_hi):
                nc.gpsimd.collective_compute(
                    "AllReduce", OP.add, replica_groups=GROUPS,
                    ins=[cc_in[o_lo * P:o_hi * P, :]],
                    outs=[cc_out[o_lo * P:o_hi * P, :]])
                for o in range(o_lo, o_hi):
                    rr = evp.tile([P, L], BF16, tag="rrb")
                    nc.sync.dma_start(out=rr, in_=cc_out[o * P:(o + 1) * P, :])
                    nc.vector.tensor_tensor(out=xT[:, o, :],
                                            in0=xT[:, o, :].bitcast(F32),
                                            in1=rr, op=OP.add)

            def evac(o, acc):
                ev = evp.tile([P, L], BF16, tag="evb")
                nc.vector.tensor_copy(out=ev, in_=acc)
                nc.sync.dma_start(out=cc_in[o * P:(o + 1) * P, :], in_=ev)
                if o == 2:
                    chunk(0, 3)
                elif o == 5:
                    chunk(3, 6)
                elif o == 7:
                    chunk(6, 8)
            return evac

        def qkv_evac(o, acc):
            nc.vector.tensor_copy(out=big[:, o, :], in_=acc)

        def relu_evac(o, acc):
            nc.scalar.activation(big[:, o, :], acc, AF.Relu)

        # ---------------- layers ----------------
        for l in range(NLAYERS):
            # ---- self-attention ----
            layernorm(xT)
            if l == 0:
                dump("dh1", hT, DK)
            proj(wqkv_sa[l], QKVO, DK, lambda o, k: hT[:, k, :], qkv_evac)
            if l == 0:
                dump("dqkv", big, QKVO)
            attention(masked=True)
            if l == 0:
                dump("dctx", ctxT, QO)
            # CA k/v depend only on the static memT: compute them during the
            # SA out-projection + AllReduce window to keep the PE busy.
            with tc.tile_pool(name="ov_ps", bufs=2, space="PSUM") as ovp:
                proj(wqkv_ca[l][4:QKVO], QKVO - 4, DK,
                     lambda o, k: memT[:, k, :],
                     lambda o, acc: qkv_evac(o + 4, acc),
                     pool=ovp, tag="kv")
                proj(wout_sa[l], DK, QO, lambda o, k: ctxT[:, k, :],
                     make_out_evac(), pool=ovp, tag="out")
            if l == 0:
                dump("dx1", xT, DK)

            # ---- cross-attention (k/v from packed embedding memT) ----
            layernorm(xT)
            if l == 0:
                dump("dh2", hT, DK)
            proj(wqkv_ca[l][0:4], 4, DK,
                 lambda o, k: hT[:, k, :], qkv_evac)
            if l == 0:
                dump("dqkv2", big, QKVO)
            attention(masked=False)
            if l == 0:
                dump("dctx2", ctxT, QO)
            proj(wout_ca[l], DK, QO, lambda o, k: ctxT[:, k, :],
                 make_out_evac())
            if l == 0:
                dump("dx2", xT, DK)

            # ---- FFN ----
            layernorm(xT)
            proj(w1_d[l], F1O, DK, lambda o, k: hT[:, k, :], relu_evac)
            if l == 0:
                dump("dff", big, F1O)
            proj(w2_d[l], DK, 2 * DK, lambda o, k: big[:, k, :],
                 make_out_evac())
            if l == 0:
                dump("dx3", xT, DK)

        # ---------------- head (vocab split across the pair) ----------------
        def head_evac(o, acc):
            for c in range(2):
                ev = prb.tile([P, 384], F32R, tag="p384")
                nc.scalar.copy(ev.bitcast(F32), acc[:, CH[c]:CH[c + 1]])
                nc.sync.dma_start(
                    out=logits[o * P:(o + 1) * P, CH[c]:CH[c + 1]],
                    in_=ev.bitcast(F32))

        proj(headw, HEADO, DK, lambda o, k: xT[:, k, :], head_evac)
        if DBG:
            nc.sync.dma_start(out=dbg["dvts"][:, :],
                              in_=vts[:, :, :, :].bitcast(F32R))
            nc.sync.dma_start(out=dbg["dinv"][:, :], in_=invb.bitcast(F32R))

    nc.finalize()
    return nc


# ---------------------------------------------------------------------------
# host side
# ---------------------------------------------------------------------------

def _pe_table(length, d):
    pos = np.arange(length, dtype=np.float32)[:, None]
    div = np.exp(np.arange(0, d, 2, dtype=np.float32) * (-np.log(10000.0) / d))
    ang = pos * div
    out = np.zeros((length, d), np.float32)
    out[:, 0::2] = np.sin(ang)
    out[:, 1::2] = np.cos(ang)
    return out


def _tp(w):
    """[..., O*128, K*128] -> [..., O, 128, K*128] pre-transposed tiles:
    out[..., o, p, k*128+c] = w[..., o*128+c, k*128+p]."""
    lead = w.shape[:-2]
    O, K = w.shape[-2] // P, w.shape[-1] // P
    w = w.reshape(*lead, O, P, K, P)
    axes = tuple(range(len(lead))) + tuple(
        len(lead) + a for a in (0, 3, 2, 1))
    return np.ascontiguousarray(
        w.transpose(*axes).reshape(*lead, O, P, K * P))


_NC_CACHE = {}
LAST_RESULT = {}


def kernel(**inputs):
    f32 = lambda a: np.ascontiguousarray(np.asarray(a, dtype=np.float32))
    text = np.asarray(inputs["text"]).astype(np.int64)
    audio = np.asarray(inputs["audio"]).astype(np.int64)
    tl = np.asarray(inputs["text_len_batch"]).astype(np.int64)
    al = np.asarray(inputs["audio_len_batch"]).astype(np.int64)
    text_table = f32(inputs["text_table"])
    audio_table = f32(inputs["audio_table"])
    sa_in_w = f32(inputs["sa_in_w"])
    sa_out_w = f32(inputs["sa_out_w"])
    ca_in_w = f32(inputs["ca_in_w"])
    ca_out_w = f32(inputs["ca_out_w"])
    ffn_w1 = f32(inputs["ffn_w1"])
    ffn_w2 = f32(inputs["ffn_w2"])

    comb = np.ascontiguousarray(np.concatenate(
        [text_table, audio_table, np.zeros((1, D), np.float32)], axis=0))
    pe_t = _pe_table(Tt, D)
    pe_a = _pe_table(Ta, D)

    in_maps = []
    for c in range(8):
        p, r = c // 2, c % 2
        tlb, alb = int(tl[p]), int(al[p])
        il = tlb + alb

        ids = np.full((L,), VT + VA, dtype=np.int64)  # default: zero row
        ids[:tlb] = text[p, :tlb]
        ids[tlb:il] = VT + audio[p, :alb]
        ids16 = np.ascontiguousarray(np.tile(ids.astype(np.int16).reshape(L // 16, 16).T, (8, 1)))

        pe_pack = np.zeros((L, D), np.float32)
        pe_pack[:tlb] = pe_t[:tlb]
        pe_pack[tlb:il] = pe_a[:alb]
        peT = np.ascontiguousarray(pe_pack.T)

        kk = np.arange(L)
        lo = np.where(kk < tlb, 0, kk).astype(np.float32)
        hi = np.where(kk < tlb, L, il).astype(np.float32)
        mlo = np.ascontiguousarray(lo.reshape(NT, P).T)          # [128, 6]
        mhi = np.ascontiguousarray((1.0 - hi).reshape(NT, P).T)

        sl = slice(512 * r, 512 * (r + 1))

        def qkv_shard(w3):
            qq = w3[:, 0:1024, :][:, sl]
            kx = w3[:, 1024:2048, :][:, sl]
            vv = w3[:, 2048:3072, :][:, sl]
            return np.ascontiguousarray(np.concatenate([qq, kx, vv], axis=1))

        hw = np.zeros((HEADO * P, D), np.float32)
        hw[0:513] = audio_table[513 * r:513 * (r + 1)]

        in_maps.append({
            "comb": comb, "ids16": ids16, "peT": peT,
            "mlo": mlo, "mhi": mhi,
            "wqkv_sa": _tp(qkv_shard(sa_in_w[:NLAYERS])),
            "wout_sa": _tp(np.ascontiguousarray(sa_out_w[:NLAYERS, :, sl])),
            "wqkv_ca": _tp(qkv_shard(ca_in_w[:NLAYERS])),
            "wout_ca": _tp(np.ascontiguousarray(ca_out_w[:NLAYERS, :, sl])),
            "w1": _tp(np.ascontiguousarray(
                ffn_w1[:NLAYERS, 2048 * r:2048 * (r + 1), :])),
            "w2": _tp(np.ascontiguousarray(
                ffn_w2[:NLAYERS, :, 2048 * r:2048 * (r + 1)])),
            "headw": _tp(hw),
        })

    if "nc" not in _NC_CACHE:
        _NC_CACHE["nc"] = _build_nc()
    nc = _NC_CACHE["nc"]
    trace = bool(int(os.environ.get("KERNEL_TRACE", "0")))
    r = run_bass_kernel_spmd(nc, in_maps, core_ids=list(range(8)), trace=trace)
    LAST_RESULT["r"] = r
    res = r.results

    out = np.empty((B, L, VA), np.float32)
    for p in range(B):
        ev = res[2 * p]["logits"]
        od = res[2 * p + 1]["logits"]
        out[p] = np.concatenate([ev[0:513], od[0:513]], axis=0).T
    return out


# revision 31
# speedup vs baseline: 1.0170x; 1.0011x over previous
"""Trainium2 Bass kernel for nn_AutoRegressive (dense transformer decoder).

Model: B=4 packed text+audio sequences, L=768, D=1024, 16 heads, DFF=4096,
6 norm-first decoder layers (self-attn w/ prefix-LM mask, cross-attn to the
packed embedding, FFN), weight-tied audio head. fp32 inputs/outputs.

Sharding: DP4 x TP2 over 8 cores. Core pair (2i, 2i+1) owns batch item i;
within a pair the 16 heads split 8+8 and DFF splits 2048+2048. Three
pair-AllReduces per layer (attn-out partials, FFN partials), each split in
two bf16 chunks overlapped with the producing projection.

Layout: activations are feature-major (x^T: [D, L], D on partitions).
Weights are pre-transposed on the host so each [128,128] stationary tile
DMAs straight into SBUF (no PE transposes, no PSUM->SBUF weight copies).
Matmuls use float32r (TF32-like e8m11, full-rate PE path). LayerNorm
partition reductions use ones-vector matmuls col-packed 4-wide; softmax
denominators come from a ones column in the AV stationary operand.
"""
import os
import numpy as np

import concourse.bass as bass
from concourse import bacc
import concourse.mybir as mybir
import concourse.tile as tile
from concourse.bass_utils import run_bass_kernel_spmd
from concourse.masks import make_identity

F32 = mybir.dt.float32
F32R = mybir.dt.float32r
BF16 = mybir.dt.bfloat16
I16 = mybir.dt.int16
AF = mybir.ActivationFunctionType
OP = mybir.AluOpType

B, Tt, Ta, L, D, H, DH, DFF, NL = 4, 128, 640, 768, 1024, 16, 64, 4096, 6
VT, VA = 256, 1026
NLAYERS = int(os.environ.get("KERNEL_NL", str(NL)))
P = 128
NT = L // P          # 6 sequence tiles
DK = D // P          # 8 feature tiles
QO = 4               # q out-tiles (local 512 dims)
QKVO = 12            # qkv out-tiles (local 1536)
F1O = 16             # ffn hidden out-tiles (local 2048)
HEADO = 5            # head out-tiles (640-row padded vocab slab)
NEG = -1.0e30
CH = (0, 384, L)
# self-attn (key-tile, query-chunk) pairs that are fully masked for every
# core: keys >= 128*tt >= 384 can never be text prefix (tl <= 128) and are
# strictly above the causal diagonal of chunk 0 (q <= 383).
SA_SKIP = {(3, 0), (4, 0), (5, 0)}


def _build_nc():
    nc = bacc.Bacc(None)

    comb = nc.declare_dram_parameter("comb", [VT + VA + 1, D], F32, isOutput=False)
    ids16 = nc.declare_dram_parameter("ids16", [P, L // 16], I16, isOutput=False)
    peT_d = nc.declare_dram_parameter("peT", [D, L], F32, isOutput=False)
    mlo_d = nc.declare_dram_parameter("mlo", [P, NT], F32, isOutput=False)
    mhi_d = nc.declare_dram_parameter("mhi", [P, NT], F32, isOutput=False)
    # pre-transposed weights: [o, p, k*128+c] = W_local[o*128+c, k*128+p]
    wqkv_sa = nc.declare_dram_parameter("wqkv_sa", [NLAYERS, QKVO, P, D], F32R, isOutput=False)
    wout_sa = nc.declare_dram_parameter("wout_sa", [NLAYERS, DK, P, 512], F32R, isOutput=False)
    wqkv_ca = nc.declare_dram_parameter("wqkv_ca", [NLAYERS, QKVO, P, D], F32R, isOutput=False)
    wout_ca = nc.declare_dram_parameter("wout_ca", [NLAYERS, DK, P, 512], F32R, isOutput=False)
    w1_d = nc.declare_dram_parameter("w1", [NLAYERS, F1O, P, D], F32R, isOutput=False)
    w2_d = nc.declare_dram_parameter("w2", [NLAYERS, DK, P, 2048], F32R, isOutput=False)
    headw = nc.declare_dram_parameter("headw", [HEADO, P, D], F32R, isOutput=False)
    logits = nc.declare_dram_parameter("logits", [HEADO * P, L], F32, isOutput=True)
    DBG = bool(int(os.environ.get("KERNEL_DEBUG", "0")))
    dbg = {}
    if DBG:
        for nm, shp in [("dx0", [D, L]), ("dmem", [D, L]), ("dh1", [D, L]),
                        ("dqkv", [1536, L]), ("dctx", [512, L]),
                        ("dx1", [D, L]), ("dh2", [D, L]),
                        ("dqkv2", [1536, L]), ("dctx2", [512, L]),
                        ("dx2", [D, L]), ("dff", [2048, L]), ("dx3", [D, L]),
                        ("dvts", [P, 512]), ("dinv", [P, L])]:
            dbg[nm] = nc.declare_dram_parameter(nm, shp, F32R, isOutput=True)

    cc_in = nc.dram_tensor("cc_in", [D, L], BF16)
    cc_out = nc.dram_tensor("cc_out", [D, L], BF16)
    GROUPS = [[0, 1], [2, 3], [4, 5], [6, 7]]

    from contextlib import ExitStack
    with tile.TileContext(nc) as tc, ExitStack() as S:
        state = S.enter_context(tc.tile_pool(name="state", bufs=1))
        wrp = S.enter_context(tc.tile_pool(name="wrp", bufs=2))
        prb = S.enter_context(tc.tile_pool(name="prb", bufs=5))
        evp = S.enter_context(tc.tile_pool(name="evp", bufs=2))

        ident = state.tile([P, P], F32)
        make_identity(nc, ident)
        identR = state.tile([P, P], F32R)
        nc.scalar.copy(identR, ident)
        ones1 = state.tile([P, 1], F32)
        nc.vector.memset(ones1, 1.0)
        onesR = state.tile([P, 1], F32R)
        nc.scalar.copy(onesR, ones1)
        onesrowR = state.tile([P, P], F32R)
        nc.vector.memset(onesrowR.bitcast(F32), 1.0)
        onesrowF = state.tile([1, P], F32)
        nc.vector.memset(onesrowF, 1.0)
        invr = state.tile([P, L], F32R)
        epst = state.tile([1, 1], F32)
        nc.vector.memset(epst, 1e-5)

        xT = state.tile([P, DK, L], F32R)
        memT = state.tile([P, DK, L], F32R)
        hT = state.tile([P, DK, L], F32R)
        big = state.tile([P, F1O, L], F32R)      # qkvT (12 slices) / h1T (16)
        ctxT = state.tile([P, QO, L], F32R)
        maskb = state.tile([P, NT, L], BF16)     # additive mask^T (0 / -1e30)
        mu_s = state.tile([1, L], F32)
        var_s = state.tile([1, L], F32)
        sd_s = state.tile([1, L], F32)
        mub = state.tile([P, L], F32)
        rsb = state.tile([P, L], F32)
        invb = mub  # disjoint lifetimes: mub lives in LN, invb in attn epilogue
        mlo_t = state.tile([P, NT], F32)
        mhi_t = state.tile([P, NT], F32)
        idx_t = state.tile([P, L // 16], I16)
        # persistent AV stationaries: [:, b, 0, 0:64]=v(hh0) col 64=ones;
        # [:, b, 1, 64:128]=v(hh1), col 0=ones (denom row 0)
        vts = state.tile([P, 2, 2, P], F32R)
        nc.vector.memset(vts.bitcast(F32), 0.0)
        for b_ in range(2):
            nc.vector.memset(vts[:, b_, 0, 64:65].bitcast(F32), 1.0)
            nc.vector.memset(vts[:, b_, 1, 0:1].bitcast(F32), 1.0)

        nc.sync.dma_start(out=mlo_t, in_=mlo_d[:, :])
        nc.sync.dma_start(out=mhi_t, in_=mhi_d[:, :])
        nc.sync.dma_start(out=idx_t, in_=ids16[:, :])

        # ---------------- mask build ----------------
        # maskb[k, q] = -1e30 * (relu(lo_k - q) + relu(q + 1 - hi_k))
        iot = mub  # staging before first LN
        nc.gpsimd.iota(iot, pattern=[[1, L]], base=0, channel_multiplier=0,
                       allow_small_or_imprecise_dtypes=True)
        with tc.tile_pool(name="mk", bufs=2) as mkp:
            for tt in range(NT):
                t1 = mkp.tile([P, L], F32, tag="mk")
                nc.scalar.activation(t1, iot, AF.Relu,
                                     bias=mlo_t[:, tt:tt + 1], scale=-1.0)
                t2 = mkp.tile([P, L], F32, tag="mk")
                nc.scalar.activation(t2, iot, AF.Relu,
                                     bias=mhi_t[:, tt:tt + 1], scale=1.0)
                nc.vector.tensor_add(out=t1, in0=t1, in1=t2)
                nc.vector.tensor_scalar_mul(maskb[:, tt, :], t1, NEG)

        # ---------------- embedding ----------------
        with tc.tile_pool(name="emb_ps", bufs=3, space="PSUM") as eps_p, \
             tc.tile_pool(name="emb_sb", bufs=2) as emb_sb:
            for tt in range(NT):
                g = emb_sb.tile([P, 1, D], F32, tag="grow")
                nc.gpsimd.dma_gather(g, comb[:, :],
                                     idx_t[:, 8 * tt:8 * (tt + 1)],
                                     num_idxs=P, num_idxs_reg=P, elem_size=D)
                for f in range(DK):
                    tp = eps_p.tile([P, P], F32, tag="tp")
                    nc.tensor.transpose(tp, g[:, 0, f * P:(f + 1) * P], ident)
                    pe_sl = emb_sb.tile([P, P], F32, tag="pe")
                    nc.sync.dma_start(
                        out=pe_sl,
                        in_=peT_d[f * P:(f + 1) * P, tt * P:(tt + 1) * P])
                    nc.vector.tensor_add(
                        out=xT[:, f, tt * P:(tt + 1) * P],
                        in0=tp, in1=pe_sl)
                    nc.scalar.copy(
                        memT[:, f, tt * P:(tt + 1) * P],
                        xT[:, f, tt * P:(tt + 1) * P].bitcast(F32))

        def dump(nm, tile_ap, n):
            if not DBG:
                return
            d = dbg[nm]
            for o in range(n):
                nc.sync.dma_start(out=d[o * P:(o + 1) * P, :],
                                  in_=tile_ap[:, o, :])

        if DBG:
            dump("dx0", xT, DK)
            dump("dmem", memT, DK)

        # ---------------- helpers ----------------
        def layernorm(src):
            """LN over partition dim of src -> hT (no affine; w=1, b=0)."""
            with tc.tile_pool(name="ln_ps", bufs=1, space="PSUM") as lps:
                s1 = [lps.tile([1, 384], F32, tag=f"s1{c}", name=f"s1{c}") for c in range(2)]
                s2 = [lps.tile([1, 384], F32, tag=f"s2{c}", name=f"s2{c}") for c in range(2)]
                for k in range(DK):
                    st, sp = (k == 0), (k == DK - 1)
                    for c in range(2):
                        sl = slice(CH[c], CH[c + 1])
                        sq = prb.tile([P, 384], F32R, tag="p384")
                        nc.vector.tensor_tensor(
                            out=sq, in0=src[:, k, sl].bitcast(F32),
                            in1=src[:, k, sl].bitcast(F32), op=OP.mult)
                        nc.tensor.matmul(s1[c], onesR, src[:, k, sl],
                                         start=st, stop=sp)
                        nc.tensor.matmul(s2[c], onesR, sq, start=st, stop=sp)
                for c in range(2):
                    sl = slice(CH[c], CH[c + 1])
                    nc.vector.tensor_scalar_mul(mu_s[:, sl], s1[c], 1.0 / D)
                    nc.vector.tensor_mul(out=var_s[:, sl], in0=mu_s[:, sl],
                                         in1=mu_s[:, sl])
                    nc.vector.scalar_tensor_tensor(
                        out=var_s[:, sl], in0=s2[c], scalar=1.0 / D,
                        in1=var_s[:, sl], op0=OP.mult, op1=OP.subtract)
            nc.scalar.activation(sd_s, var_s, AF.Sqrt, bias=epst[0:1, 0:1])
            nc.vector.reciprocal_approx_fast(out=sd_s, in_=sd_s)
            nc.gpsimd.partition_broadcast(mub, mu_s[0:1, :])
            nc.gpsimd.partition_broadcast(rsb, sd_s[0:1, :])
            for k in range(DK):
                nc.vector.tensor_tensor(out=hT[:, k, :],
                                        in0=src[:, k, :].bitcast(F32),
                                        in1=mub, op=OP.subtract)
                nc.vector.tensor_mul(out=hT[:, k, :],
                                     in0=hT[:, k, :].bitcast(F32), in1=rsb)

        def proj(w_ap, n_o, kcnt, rhs_fn, out_fn, pool=None, tag="acc"):
            """acc[o] = sum_k W^T[o,k] @ rhs(o,k); out_fn(o, acc_psum).

            w_ap: DRAM AP [n_o, 128, kcnt*128], pre-transposed tiles.
            """
            from contextlib import nullcontext
            cm = (nullcontext(pool) if pool is not None
                  else tc.tile_pool(name="pj_ps", bufs=2, space="PSUM"))
            with cm as pps:
                for o in range(n_o):
                    wslab = wrp.tile([P, F1O * P], F32R, tag="wslab")
                    half = kcnt * P // 2
                    nc.sync.dma_start(out=wslab[:, 0:half],
                                      in_=w_ap[o, :, 0:half])
                    nc.sync.dma_start(out=wslab[:, half:kcnt * P],
                                      in_=w_ap[o, :, half:kcnt * P])
                    acc = pps.tile([P, L], F32, tag=tag)
                    for k in range(kcnt):
                        wT = wslab[:, k * P:(k + 1) * P]
                        rhs = rhs_fn(o, k)
                        st, sp = (k == 0), (k == kcnt - 1)
                        nc.tensor.matmul(acc[:, 0:512], wT, rhs[:, 0:512],
                                         start=st, stop=sp)
                        nc.tensor.matmul(acc[:, 512:L], wT, rhs[:, 512:L],
                                         start=st, stop=sp)
                    out_fn(o, acc)

        def attention(masked):
            """big[:, 0:12] = qkvT (2 heads per 128-tile) -> ctxT."""
            skip = SA_SKIP if masked else set()
            cmax = {c: max(tt for tt in range(NT) if (tt, c) not in skip)
                    for c in range(2)}
            cmin = {c: min(tt for tt in range(NT) if (tt, c) not in skip)
                    for c in range(2)}
            with tc.tile_pool(name="at_sps", bufs=2, space="PSUM") as sps, \
                 tc.tile_pool(name="at_cps", bufs=4, space="PSUM") as cps, \
                 tc.tile_pool(name="at_vps", bufs=2, space="PSUM") as vps:
                for j in range(QO):          # head pair j: heads 2j, 2j+1
                    ctx = [[cps.tile([P, 384], F32, tag="ctx",
                                          name=f"ctx{hh}{c}")
                            for c in range(2)] for hh in range(2)]
                    for tt in range(NT):
                        prob = [[None, None], [None, None]]
                        for hh in range(2):
                            hb = 64 * hh
                            kT = big[hb:hb + 64, 4 + j, tt * P:(tt + 1) * P]
                            for c in range(2):
                                if (tt, c) in skip:
                                    continue
                                sc = sps.tile([P, 384], F32, tag="sc")
                                qT = big[hb:hb + 64, j, CH[c]:CH[c + 1]]
                                nc.tensor.matmul(sc, kT, qT,
                                                 start=True, stop=True)
                                if masked:
                                    nc.vector.scalar_tensor_tensor(
                                        out=sc,
                                        in0=maskb[:, tt, CH[c]:CH[c + 1]],
                                        scalar=1.0, in1=sc,
                                        op0=OP.mult, op1=OP.add)
                                pr = prb.tile([P, 384], F32R, tag="p384")
                                nc.scalar.activation(pr, sc, AF.Exp,
                                                     scale=0.125)
                                prob[hh][c] = pr
                        for hh in range(2):
                            hb = 64 * hh
                            # v_tok: transpose vT slice [64, 128] -> [128, 64]
                            vtp = vps.tile([P, 64], F32R, tag="vtp")
                            nc.tensor.transpose(
                                vtp,
                                big[hb:hb + 64, 8 + j, tt * P:(tt + 1) * P],
                                identR[hb:hb + 64, hb:hb + 64])
                            vsl = slice(0, 64) if hh == 0 else slice(64, 128)
                            nc.scalar.copy(vts[:, tt % 2, hh, vsl],
                                           vtp.bitcast(F32))
                            lhs = (vts[:, tt % 2, 0, 0:65] if hh == 0
                                   else vts[:, tt % 2, 1, 0:128])
                            m_sl = slice(0, 65) if hh == 0 else slice(0, 128)
                            for c in range(2):
                                if (tt, c) in skip:
                                    continue
                                nc.tensor.matmul(
                                    ctx[hh][c][m_sl, :], lhs, prob[hh][c],
                                    start=(tt == cmin[c]),
                                    stop=(tt == cmax[c]))
                    # epilogue: first evacuate each ctx PSUM tile to SBUF
                    # with one DVE copy so the pool frees for the next head
                    # pair's AV immediately; then normalize from SBUF. The
                    # approx reciprocal only works at partition base 0, so
                    # hh0's denominator (row 64) moves there via a 1-column
                    # PE matmul first.
                    # hT is dead during attention (consumed by the QKV
                    # projection, rewritten by the next LN) - use its slices
                    # as the evacuation target.
                    cs = [[None, None], [None, None]]
                    for hh in range(2):
                        for c in range(2):
                            t = hT[:, (0 if hh == 0 else 4) + j,
                                   CH[c]:CH[c] + 384]
                            nc.vector.tensor_copy(out=t, in_=ctx[hh][c])
                            cs[hh][c] = t
                    for hh in range(2):
                        hb = 64 * hh
                        dr = 64 if hh == 0 else 0
                        for c in range(2):
                            sl = slice(CH[c], CH[c + 1])
                            if hh == 0:
                                db = sps.tile([P, 384], F32, tag="sc",
                                              name=f"db{c}")
                                nc.tensor.matmul(db[0:1, :],
                                                 onesrowR[dr:dr + 1, 0:1],
                                                 cs[0][c][dr:dr + 1, :],
                                                 start=True, stop=True)
                                den0 = db[0:1, :]
                            else:
                                den0 = cs[1][c][0:1, :].bitcast(F32)
                            t1 = prb.tile([P, 384], F32R, tag="p384")
                            nc.vector.reciprocal_approx_fast(
                                out=t1[0:1, :].bitcast(F32), in_=den0)
                            ib = sps.tile([P, 384], F32, tag="sc",
                                          name=f"ib{hh}{c}")
                            nc.tensor.matmul(ib, onesrowF[0:1, :],
                                             t1[0:1, :].bitcast(F32),
                                             start=True, stop=True)
                            nc.vector.tensor_mul(
                                out=ctxT[hb:hb + 64, j, sl],
                                in0=cs[hh][c][hb:hb + 64, :].bitcast(F32),
                                in1=ib[hb:hb + 64, :])

        def make_out_evac():
            """Evacuate out-proj partials as bf16, AllReduce in two chunks
            overlapped with the second half of the projection, and add the
            reduced result back into xT."""
            def chunk(o_lo, o_hi):
                nc.gpsimd.collective_compute(
                    "AllReduce", OP.add, replica_groups=GROUPS,
                    ins=[cc_in[o_lo * P:o_hi * P, :]],
                    outs=[cc_out[o_lo * P:o_hi * P, :]])
                for o in range(o_lo, o_hi):
                    rr = evp.tile([P, L], BF16, tag="rrb")
                    nc.sync.dma_start(out=rr, in_=cc_out[o * P:(o + 1) * P, :])
                    nc.vector.tensor_tensor(out=xT[:, o, :],
                                            in0=xT[:, o, :].bitcast(F32),
                                            in1=rr, op=OP.add)

            def evac(o, acc):
                ev = evp.tile([P, L], BF16, tag="evb")
                nc.vector.tensor_copy(out=ev, in_=acc)
                nc.sync.dma_start(out=cc_in[o * P:(o + 1) * P, :], in_=ev)
                if o == 2:
                    chunk(0, 3)
                elif o == 5:
                    chunk(3, 6)
                elif o == 7:
                    chunk(6, 8)
            return evac

        def qkv_evac(o, acc):
            nc.vector.tensor_copy(out=big[:, o, :], in_=acc)

        def relu_evac(o, acc):
            nc.scalar.activation(big[:, o, :], acc, AF.Relu)

        # ---------------- layers ----------------
        for l in range(NLAYERS):
            # ---- self-attention ----
            layernorm(xT)
            if l == 0:
                dump("dh1", hT, DK)
            proj(wqkv_sa[l], QKVO, DK, lambda o, k: hT[:, k, :], qkv_evac)
            if l == 0:
                dump("dqkv", big, QKVO)
            attention(masked=True)
            if l == 0:
                dump("dctx", ctxT, QO)
            # CA k/v depend only on the static memT: compute them during the
            # SA out-projection + AllReduce window to keep the PE busy.
            with tc.tile_pool(name="ov_ps", bufs=2, space="PSUM") as ovp:
                proj(wqkv_ca[l][4:QKVO], QKVO - 4, DK,
                     lambda o, k: memT[:, k, :],
                     lambda o, acc: qkv_evac(o + 4, acc),
                     pool=ovp, tag="kv")
                proj(wout_sa[l], DK, QO, lambda o, k: ctxT[:, k, :],
                     make_out_evac(), pool=ovp, tag="out")
            if l == 0:
                dump("dx1", xT, DK)

            # ---- cross-attention (k/v from packed embedding memT) ----
            layernorm(xT)
            if l == 0:
                dump("dh2", hT, DK)
            proj(wqkv_ca[l][0:4], 4, DK,
                 lambda o, k: hT[:, k, :], qkv_evac)
            if l == 0:
                dump("dqkv2", big, QKVO)
            attention(masked=False)
            if l == 0:
                dump("dctx2", ctxT, QO)
            proj(wout_ca[l], DK, QO, lambda o, k: ctxT[:, k, :],
                 make_out_evac())
            if l == 0:
                dump("dx2", xT, DK)

            # ---- FFN ----
            layernorm(xT)
            proj(w1_d[l], F1O, DK, lambda o, k: hT[:, k, :], relu_evac)
            if l == 0:
                dump("dff", big, F1O)
            proj(w2_d[l], DK, 2 * DK, lambda o, k: big[:, k, :],
                 make_out_evac())
            if l == 0:
                dump("dx3", xT, DK)

        # ---------------- head (vocab split across the pair) ----------------
        def head_evac(o, acc):
            for c in range(2):
                ev = prb.tile([P, 384], F32R, tag="p384")
                nc.scalar.copy(ev.bitcast(F32), acc[:, CH[c]:CH[c + 1]])
                nc.sync.dma_start(
                    out=logits[o * P:(o + 1) * P, CH[c]:CH[c + 1]],
                    in_=ev.bitcast(F32))

        proj(headw, HEADO, DK, lambda o, k: xT[:, k, :], head_evac)
        if DBG:
            nc.sync.dma_start(out=dbg["dvts"][:, :],
                              in_=vts[:, :, :, :].bitcast(F32R))
            nc.sync.dma_start(out=dbg["dinv"][:, :], in_=invb.bitcast(F32R))

    nc.finalize()
    return nc


# ---------------------------------------------------------------------------
# host side
# ---------------------------------------------------------------------------

def _pe_table(length, d):
    pos = np.arange(length, dtype=np.float32)[:, None]
    div = np.exp(np.arange(0, d, 2, dtype=np.float32) * (-np.log(10000.0) / d))
    ang = pos * div
    out = np.zeros((length, d), np.float32)
    out[:, 0::2] = np.sin(ang)
    out[:, 1::2] = np.cos(ang)
    return out


def _tp(w):
    """[..., O*128, K*128] -> [..., O, 128, K*128] pre-transposed tiles:
    out[..., o, p, k*128+c] = w[..., o*128+c, k*128+p]."""
    lead = w.shape[:-2]
    O, K = w.shape[-2] // P, w.shape[-1] // P
    w = w.reshape(*lead, O, P, K, P)
    axes = tuple(range(len(lead))) + tuple(
        len(lead) + a for a in (0, 3, 2, 1))
    return np.ascontiguousarray(
        w.transpose(*axes).reshape(*lead, O, P, K * P))


_NC_CACHE = {}
LAST_RESULT = {}


def kernel(**inputs):
    f32 = lambda a: np.ascontiguousarray(np.asarray(a, dtype=np.float32))
    text = np.asarray(inputs["text"]).astype(np.int64)
    audio = np.asarray(inputs["audio"]).astype(np.int64)
    tl = np.asarray(inputs["text_len_batch"]).astype(np.int64)
    al = np.asarray(inputs["audio_len_batch"]).astype(np.int64)
    text_table = f32(inputs["text_table"])
    audio_table = f32(inputs["audio_table"])
    sa_in_w = f32(inputs["sa_in_w"])
    sa_out_w = f32(inputs["sa_out_w"])
    ca_in_w = f32(inputs["ca_in_w"])
    ca_out_w = f32(inputs["ca_out_w"])
    ffn_w1 = f32(inputs["ffn_w1"])
    ffn_w2 = f32(inputs["ffn_w2"])

    comb = np.ascontiguousarray(np.concatenate(
        [text_table, audio_table, np.zeros((1, D), np.float32)], axis=0))
    pe_t = _pe_table(Tt, D)
    pe_a = _pe_table(Ta, D)

    in_maps = []
    for c in range(8):
        p, r = c // 2, c % 2
        tlb, alb = int(tl[p]), int(al[p])
        il = tlb + alb

        ids = np.full((L,), VT + VA, dtype=np.int64)  # default: zero row
        ids[:tlb] = text[p, :tlb]
        ids[tlb:il] = VT + audio[p, :alb]
        ids16 = np.ascontiguousarray(np.tile(ids.astype(np.int16).reshape(L // 16, 16).T, (8, 1)))

        pe_pack = np.zeros((L, D), np.float32)
        pe_pack[:tlb] = pe_t[:tlb]
        pe_pack[tlb:il] = pe_a[:alb]
        peT = np.ascontiguousarray(pe_pack.T)

        kk = np.arange(L)
        lo = np.where(kk < tlb, 0, kk).astype(np.float32)
        hi = np.where(kk < tlb, L, il).astype(np.float32)
        mlo = np.ascontiguousarray(lo.reshape(NT, P).T)          # [128, 6]
        mhi = np.ascontiguousarray((1.0 - hi).reshape(NT, P).T)

        sl = slice(512 * r, 512 * (r + 1))

        def qkv_shard(w3):
            qq = w3[:, 0:1024, :][:, sl]
            kx = w3[:, 1024:2048, :][:, sl]
            vv = w3[:, 2048:3072, :][:, sl]
            return np.ascontiguousarray(np.concatenate([qq, kx, vv], axis=1))

        hw = np.zeros((HEADO * P, D), np.float32)
        hw[0:513] = audio_table[513 * r:513 * (r + 1)]

        in_maps.append({
            "comb": comb, "ids16": ids16, "peT": peT,
            "mlo": mlo, "mhi": mhi,
            "wqkv_sa": _tp(qkv_shard(sa_in_w[:NLAYERS])),
            "wout_sa": _tp(np.ascontiguousarray(sa_out_w[:NLAYERS, :, sl])),
            "wqkv_ca": _tp(qkv_shard(ca_in_w[:NLAYERS])),
            "wout_ca": _tp(np.ascontiguousarray(ca_out_w[:NLAYERS, :, sl])),
            "w1": _tp(np.ascontiguousarray(
                ffn_w1[:NLAYERS, 2048 * r:2048 * (r + 1), :])),
            "w2": _tp(np.ascontiguousarray(
                ffn_w2[:NLAYERS, :, 2048 * r:2048 * (r + 1)])),
            "headw": _tp(hw),
        })

    if "nc" not in _NC_CACHE:
        _NC_CACHE["nc"] = _build_nc()
    nc = _NC_CACHE["nc"]
    trace = bool(int(os.environ.get("KERNEL_TRACE", "0")))
    r = run_bass_kernel_spmd(nc, in_maps, core_ids=list(range(8)), trace=trace)
    LAST_RESULT["r"] = r
    res = r.results

    out = np.empty((B, L, VA), np.float32)
    for p in range(B):
        ev = res[2 * p]["logits"]
        od = res[2 * p + 1]["logits"]
        out[p] = np.concatenate([ev[0:513], od[0:513]], axis=0).T
    return out


# revision 33
# speedup vs baseline: 1.0329x; 1.0156x over previous
"""Trainium2 Bass kernel for nn_AutoRegressive (dense transformer decoder).

Model: B=4 packed text+audio sequences, L=768, D=1024, 16 heads, DFF=4096,
6 norm-first decoder layers (self-attn w/ prefix-LM mask, cross-attn to the
packed embedding, FFN), weight-tied audio head. fp32 inputs/outputs.

Sharding: DP4 x TP2 over 8 cores. Core pair (2i, 2i+1) owns batch item i;
within a pair the 16 heads split 8+8 and DFF splits 2048+2048. Three
pair-AllReduces per layer (attn-out partials, FFN partials), each split in
two bf16 chunks overlapped with the producing projection.

Layout: activations are feature-major (x^T: [D, L], D on partitions).
Weights are pre-transposed on the host so each [128,128] stationary tile
DMAs straight into SBUF (no PE transposes, no PSUM->SBUF weight copies).
Matmuls use float32r (TF32-like e8m11, full-rate PE path). LayerNorm
partition reductions use ones-vector matmuls col-packed 4-wide; softmax
denominators come from a ones column in the AV stationary operand.
"""
import os
import numpy as np

import concourse.bass as bass
from concourse import bacc
import concourse.mybir as mybir
import concourse.tile as tile
from concourse.bass_utils import run_bass_kernel_spmd
from concourse.masks import make_identity

F32 = mybir.dt.float32
F32R = mybir.dt.float32r
BF16 = mybir.dt.bfloat16
I16 = mybir.dt.int16
AF = mybir.ActivationFunctionType
OP = mybir.AluOpType

B, Tt, Ta, L, D, H, DH, DFF, NL = 4, 128, 640, 768, 1024, 16, 64, 4096, 6
VT, VA = 256, 1026
NLAYERS = int(os.environ.get("KERNEL_NL", str(NL)))
P = 128
NT = L // P          # 6 sequence tiles
DK = D // P          # 8 feature tiles
QO = 4               # q out-tiles (local 512 dims)
QKVO = 12            # qkv out-tiles (local 1536)
F1O = 16             # ffn hidden out-tiles (local 2048)
HEADO = 5            # head out-tiles (640-row padded vocab slab)
NEG = -1.0e30
CH = (0, 384, L)
# self-attn (key-tile, query-chunk) pairs that are fully masked for every
# core: keys >= 128*tt >= 384 can never be text prefix (tl <= 128) and are
# strictly above the causal diagonal of chunk 0 (q <= 383).
SA_SKIP = {(3, 0), (4, 0), (5, 0)}


def _build_nc():
    nc = bacc.Bacc(None)

    comb = nc.declare_dram_parameter("comb", [VT + VA + 1, D], F32, isOutput=False)
    ids16 = nc.declare_dram_parameter("ids16", [P, L // 16], I16, isOutput=False)
    peT_d = nc.declare_dram_parameter("peT", [D, L], F32, isOutput=False)
    mlo_d = nc.declare_dram_parameter("mlo", [P, NT], F32, isOutput=False)
    mhi_d = nc.declare_dram_parameter("mhi", [P, NT], F32, isOutput=False)
    # pre-transposed weights: [o, p, k*128+c] = W_local[o*128+c, k*128+p]
    wqkv_sa = nc.declare_dram_parameter("wqkv_sa", [NLAYERS, QKVO, P, D], F32R, isOutput=False)
    wout_sa = nc.declare_dram_parameter("wout_sa", [NLAYERS, DK, P, 512], F32R, isOutput=False)
    wqkv_ca = nc.declare_dram_parameter("wqkv_ca", [NLAYERS, QKVO, P, D], F32R, isOutput=False)
    wout_ca = nc.declare_dram_parameter("wout_ca", [NLAYERS, DK, P, 512], F32R, isOutput=False)
    w1_d = nc.declare_dram_parameter("w1", [NLAYERS, F1O, P, D], F32R, isOutput=False)
    w2_d = nc.declare_dram_parameter("w2", [NLAYERS, DK, P, 2048], F32R, isOutput=False)
    headw = nc.declare_dram_parameter("headw", [HEADO, P, D], F32R, isOutput=False)
    logits = nc.declare_dram_parameter("logits", [HEADO * P, L], F32, isOutput=True)
    DBG = bool(int(os.environ.get("KERNEL_DEBUG", "0")))
    dbg = {}
    if DBG:
        for nm, shp in [("dx0", [D, L]), ("dmem", [D, L]), ("dh1", [D, L]),
                        ("dqkv", [1536, L]), ("dctx", [512, L]),
                        ("dx1", [D, L]), ("dh2", [D, L]),
                        ("dqkv2", [1536, L]), ("dctx2", [512, L]),
                        ("dx2", [D, L]), ("dff", [2048, L]), ("dx3", [D, L]),
                        ("dvts", [P, 512]), ("dinv", [P, L])]:
            dbg[nm] = nc.declare_dram_parameter(nm, shp, F32R, isOutput=True)

    cc_in = nc.dram_tensor("cc_in", [D, L], BF16)
    cc_out = nc.dram_tensor("cc_out", [D, L], BF16)
    GROUPS = [[0, 1], [2, 3], [4, 5], [6, 7]]

    from contextlib import ExitStack
    with tile.TileContext(nc) as tc, ExitStack() as S:
        state = S.enter_context(tc.tile_pool(name="state", bufs=1))
        wrp = S.enter_context(tc.tile_pool(name="wrp", bufs=2))
        prb = S.enter_context(tc.tile_pool(name="prb", bufs=5))
        evp = S.enter_context(tc.tile_pool(name="evp", bufs=2))

        ident = state.tile([P, P], F32)
        make_identity(nc, ident)
        identR = state.tile([P, P], F32R)
        nc.scalar.copy(identR, ident)
        ones1 = state.tile([P, 1], F32)
        nc.vector.memset(ones1, 1.0)
        onesR = state.tile([P, 1], F32R)
        nc.scalar.copy(onesR, ones1)
        onesrowR = state.tile([P, P], F32R)
        nc.vector.memset(onesrowR.bitcast(F32), 1.0)
        onesrowF = state.tile([1, P], F32)
        nc.vector.memset(onesrowF, 1.0)
        invr = state.tile([P, L], F32R)
        epst = state.tile([1, 1], F32)
        nc.vector.memset(epst, 1e-5)

        xT = state.tile([P, DK, L], F32R)
        memT = state.tile([P, DK, L], F32R)
        hT = state.tile([P, DK, L], F32R)
        big = state.tile([P, F1O, L], F32R)      # qkvT (12 slices) / h1T (16)
        ctxT = state.tile([P, QO, L], F32R)
        maskb = state.tile([P, NT, L], BF16)     # additive mask^T (0 / -1e30)
        mu_s = state.tile([1, L], F32)
        var_s = state.tile([1, L], F32)
        sd_s = state.tile([1, L], F32)
        mub = state.tile([P, L], F32)
        rsb = state.tile([P, L], F32)
        invb = mub  # disjoint lifetimes: mub lives in LN, invb in attn epilogue
        mlo_t = state.tile([P, NT], F32)
        mhi_t = state.tile([P, NT], F32)
        idx_t = state.tile([P, L // 16], I16)
        # persistent AV stationaries: [:, b, 0, 0:64]=v(hh0) col 64=ones;
        # [:, b, 1, 64:128]=v(hh1), col 0=ones (denom row 0)
        vts = state.tile([P, 2, 2, P], F32R)
        nc.vector.memset(vts.bitcast(F32), 0.0)
        for b_ in range(2):
            nc.vector.memset(vts[:, b_, 0, 64:65].bitcast(F32), 1.0)
            nc.vector.memset(vts[:, b_, 1, 0:1].bitcast(F32), 1.0)

        nc.sync.dma_start(out=mlo_t, in_=mlo_d[:, :])
        nc.sync.dma_start(out=mhi_t, in_=mhi_d[:, :])
        nc.sync.dma_start(out=idx_t, in_=ids16[:, :])

        # ---------------- mask build ----------------
        # maskb[k, q] = -1e30 * (relu(lo_k - q) + relu(q + 1 - hi_k))
        iot = mub  # staging before first LN
        nc.gpsimd.iota(iot, pattern=[[1, L]], base=0, channel_multiplier=0,
                       allow_small_or_imprecise_dtypes=True)
        with tc.tile_pool(name="mk", bufs=2) as mkp:
            for tt in range(NT):
                t1 = mkp.tile([P, L], F32, tag="mk")
                nc.scalar.activation(t1, iot, AF.Relu,
                                     bias=mlo_t[:, tt:tt + 1], scale=-1.0)
                t2 = mkp.tile([P, L], F32, tag="mk")
                nc.scalar.activation(t2, iot, AF.Relu,
                                     bias=mhi_t[:, tt:tt + 1], scale=1.0)
                nc.vector.tensor_add(out=t1, in0=t1, in1=t2)
                nc.vector.tensor_scalar_mul(maskb[:, tt, :], t1, NEG)

        # ---------------- embedding ----------------
        with tc.tile_pool(name="emb_ps", bufs=3, space="PSUM") as eps_p, \
             tc.tile_pool(name="emb_sb", bufs=2) as emb_sb:
            for tt in range(NT):
                g = emb_sb.tile([P, 1, D], F32, tag="grow")
                nc.gpsimd.dma_gather(g, comb[:, :],
                                     idx_t[:, 8 * tt:8 * (tt + 1)],
                                     num_idxs=P, num_idxs_reg=P, elem_size=D)
                for f in range(DK):
                    tp = eps_p.tile([P, P], F32, tag="tp")
                    nc.tensor.transpose(tp, g[:, 0, f * P:(f + 1) * P], ident)
                    pe_sl = emb_sb.tile([P, P], F32, tag="pe")
                    nc.sync.dma_start(
                        out=pe_sl,
                        in_=peT_d[f * P:(f + 1) * P, tt * P:(tt + 1) * P])
                    nc.vector.tensor_add(
                        out=xT[:, f, tt * P:(tt + 1) * P],
                        in0=tp, in1=pe_sl)
                    nc.scalar.copy(
                        memT[:, f, tt * P:(tt + 1) * P],
                        xT[:, f, tt * P:(tt + 1) * P].bitcast(F32))

        def dump(nm, tile_ap, n):
            if not DBG:
                return
            d = dbg[nm]
            for o in range(n):
                nc.sync.dma_start(out=d[o * P:(o + 1) * P, :],
                                  in_=tile_ap[:, o, :])

        if DBG:
            dump("dx0", xT, DK)
            dump("dmem", memT, DK)

        # ---------------- helpers ----------------
        def layernorm(src):
            """LN over partition dim of src -> hT (no affine; w=1, b=0)."""
            with tc.tile_pool(name="ln_ps", bufs=1, space="PSUM") as lps:
                s1 = [lps.tile([1, 384], F32, tag=f"s1{c}", name=f"s1{c}") for c in range(2)]
                s2 = [lps.tile([1, 384], F32, tag=f"s2{c}", name=f"s2{c}") for c in range(2)]
                for k in range(DK):
                    st, sp = (k == 0), (k == DK - 1)
                    for c in range(2):
                        sl = slice(CH[c], CH[c + 1])
                        sq = prb.tile([P, 384], F32R, tag="p384")
                        nc.vector.tensor_tensor(
                            out=sq, in0=src[:, k, sl].bitcast(F32),
                            in1=src[:, k, sl].bitcast(F32), op=OP.mult)
                        nc.tensor.matmul(s1[c], onesR, src[:, k, sl],
                                         start=st, stop=sp)
                        nc.tensor.matmul(s2[c], onesR, sq, start=st, stop=sp)
                for c in range(2):
                    sl = slice(CH[c], CH[c + 1])
                    nc.vector.tensor_scalar_mul(mu_s[:, sl], s1[c], 1.0 / D)
                    nc.vector.tensor_mul(out=var_s[:, sl], in0=mu_s[:, sl],
                                         in1=mu_s[:, sl])
                    nc.vector.scalar_tensor_tensor(
                        out=var_s[:, sl], in0=s2[c], scalar=1.0 / D,
                        in1=var_s[:, sl], op0=OP.mult, op1=OP.subtract)
            nc.scalar.activation(sd_s, var_s, AF.Sqrt, bias=epst[0:1, 0:1])
            nc.vector.reciprocal_approx_fast(out=sd_s, in_=sd_s)
            nc.gpsimd.partition_broadcast(mub, mu_s[0:1, :])
            nc.gpsimd.partition_broadcast(rsb, sd_s[0:1, :])
            for k in range(DK):
                nc.vector.tensor_tensor(out=hT[:, k, :],
                                        in0=src[:, k, :].bitcast(F32),
                                        in1=mub, op=OP.subtract)
                nc.vector.tensor_mul(out=hT[:, k, :],
                                     in0=hT[:, k, :].bitcast(F32), in1=rsb)

        def proj(w_ap, n_o, kcnt, rhs_fn, out_fn, pool=None, tag="acc"):
            """acc[o] = sum_k W^T[o,k] @ rhs(o,k); out_fn(o, acc_psum).

            w_ap: DRAM AP [n_o, 128, kcnt*128], pre-transposed tiles.
            """
            from contextlib import nullcontext
            cm = (nullcontext(pool) if pool is not None
                  else tc.tile_pool(name="pj_ps", bufs=2, space="PSUM"))
            with cm as pps:
                for o in range(n_o):
                    wslab = wrp.tile([P, F1O * P], F32R, tag="wslab")
                    half = kcnt * P // 2
                    nc.sync.dma_start(out=wslab[:, 0:half],
                                      in_=w_ap[o, :, 0:half])
                    nc.sync.dma_start(out=wslab[:, half:kcnt * P],
                                      in_=w_ap[o, :, half:kcnt * P])
                    acc = pps.tile([P, L], F32, tag=tag)
                    for k in range(kcnt):
                        wT = wslab[:, k * P:(k + 1) * P]
                        rhs = rhs_fn(o, k)
                        st, sp = (k == 0), (k == kcnt - 1)
                        nc.tensor.matmul(acc[:, 0:512], wT, rhs[:, 0:512],
                                         start=st, stop=sp)
                        nc.tensor.matmul(acc[:, 512:L], wT, rhs[:, 512:L],
                                         start=st, stop=sp)
                    out_fn(o, acc)

        def attention(masked):
            """big[:, 0:12] = qkvT (2 heads per 128-tile) -> ctxT."""
            skip = SA_SKIP if masked else set()
            cmax = {c: max(tt for tt in range(NT) if (tt, c) not in skip)
                    for c in range(2)}
            cmin = {c: min(tt for tt in range(NT) if (tt, c) not in skip)
                    for c in range(2)}
            with tc.tile_pool(name="at_sps", bufs=3, space="PSUM") as sps, \
                 tc.tile_pool(name="at_cps", bufs=4, space="PSUM") as cps, \
                 tc.tile_pool(name="at_vps", bufs=1, space="PSUM") as vps:
                for j in range(QO):          # head pair j: heads 2j, 2j+1
                    ctx = [[cps.tile([P, 384], F32, tag="ctx",
                                          name=f"ctx{hh}{c}")
                            for c in range(2)] for hh in range(2)]
                    for tt in range(NT):
                        prob = [[None, None], [None, None]]
                        for hh in range(2):
                            hb = 64 * hh
                            kT = big[hb:hb + 64, 4 + j, tt * P:(tt + 1) * P]
                            for c in range(2):
                                if (tt, c) in skip:
                                    continue
                                sc = sps.tile([P, 384], F32, tag="sc")
                                qT = big[hb:hb + 64, j, CH[c]:CH[c + 1]]
                                nc.tensor.matmul(sc, kT, qT,
                                                 start=True, stop=True)
                                if masked:
                                    nc.vector.scalar_tensor_tensor(
                                        out=sc,
                                        in0=maskb[:, tt, CH[c]:CH[c + 1]],
                                        scalar=1.0, in1=sc,
                                        op0=OP.mult, op1=OP.add)
                                pr = prb.tile([P, 384], F32R, tag="p384")
                                nc.scalar.activation(pr, sc, AF.Exp,
                                                     scale=0.125)
                                prob[hh][c] = pr
                        for hh in range(2):
                            hb = 64 * hh
                            # v_tok: transpose vT slice [64, 128] -> [128, 64]
                            vtp = vps.tile([P, 64], F32R, tag="vtp")
                            nc.tensor.transpose(
                                vtp,
                                big[hb:hb + 64, 8 + j, tt * P:(tt + 1) * P],
                                identR[hb:hb + 64, hb:hb + 64])
                            vsl = slice(0, 64) if hh == 0 else slice(64, 128)
                            nc.scalar.copy(vts[:, tt % 2, hh, vsl],
                                           vtp.bitcast(F32))
                            lhs = (vts[:, tt % 2, 0, 0:65] if hh == 0
                                   else vts[:, tt % 2, 1, 0:128])
                            m_sl = slice(0, 65) if hh == 0 else slice(0, 128)
                            for c in range(2):
                                if (tt, c) in skip:
                                    continue
                                nc.tensor.matmul(
                                    ctx[hh][c][m_sl, :], lhs, prob[hh][c],
                                    start=(tt == cmin[c]),
                                    stop=(tt == cmax[c]))
                    # epilogue: first evacuate each ctx PSUM tile to SBUF
                    # with one DVE copy so the pool frees for the next head
                    # pair's AV immediately; then normalize from SBUF. The
                    # approx reciprocal only works at partition base 0, so
                    # hh0's denominator (row 64) moves there via a 1-column
                    # PE matmul first.
                    # hT is dead during attention (consumed by the QKV
                    # projection, rewritten by the next LN) - use its slices
                    # as the evacuation target.
                    cs = [[None, None], [None, None]]
                    for hh in range(2):
                        for c in range(2):
                            t = hT[:, (0 if hh == 0 else 4) + j,
                                   CH[c]:CH[c] + 384]
                            nc.vector.tensor_copy(out=t, in_=ctx[hh][c])
                            cs[hh][c] = t
                    for hh in range(2):
                        hb = 64 * hh
                        dr = 64 if hh == 0 else 0
                        for c in range(2):
                            sl = slice(CH[c], CH[c + 1])
                            if hh == 0:
                                db = sps.tile([P, 384], F32, tag="sc",
                                              name=f"db{c}")
                                nc.tensor.matmul(db[0:1, :],
                                                 onesrowR[dr:dr + 1, 0:1],
                                                 cs[0][c][dr:dr + 1, :],
                                                 start=True, stop=True)
                                den0 = db[0:1, :]
                            else:
                                den0 = cs[1][c][0:1, :].bitcast(F32)
                            t1 = prb.tile([P, 384], F32R, tag="p384")
                            nc.vector.reciprocal_approx_fast(
                                out=t1[0:1, :].bitcast(F32), in_=den0)
                            ib = sps.tile([P, 384], F32, tag="sc",
                                          name=f"ib{hh}{c}")
                            nc.tensor.matmul(ib, onesrowF[0:1, :],
                                             t1[0:1, :].bitcast(F32),
                                             start=True, stop=True)
                            nc.vector.tensor_mul(
                                out=ctxT[hb:hb + 64, j, sl],
                                in0=cs[hh][c][hb:hb + 64, :].bitcast(F32),
                                in1=ib[hb:hb + 64, :])

        def make_out_evac():
            """Evacuate out-proj partials as bf16, AllReduce in two chunks
            overlapped with the second half of the projection, and add the
            reduced result back into xT."""
            def chunk(o_lo, o_hi):
                nc.gpsimd.collective_compute(
                    "AllReduce", OP.add, replica_groups=GROUPS,
                    ins=[cc_in[o_lo * P:o_hi * P, :]],
                    outs=[cc_out[o_lo * P:o_hi * P, :]])
                for o in range(o_lo, o_hi):
                    rr = evp.tile([P, L], BF16, tag="rrb")
                    nc.sync.dma_start(out=rr, in_=cc_out[o * P:(o + 1) * P, :])
                    nc.vector.tensor_tensor(out=xT[:, o, :],
                                            in0=xT[:, o, :].bitcast(F32),
                                            in1=rr, op=OP.add)

            def evac(o, acc):
                ev = evp.tile([P, L], BF16, tag="evb")
                nc.vector.tensor_copy(out=ev, in_=acc)
                nc.sync.dma_start(out=cc_in[o * P:(o + 1) * P, :], in_=ev)
                if o == 2:
                    chunk(0, 3)
                elif o == 5:
                    chunk(3, 6)
                elif o == 7:
                    chunk(6, 8)
            return evac

        def qkv_evac(o, acc):
            nc.vector.tensor_copy(out=big[:, o, :], in_=acc)

        def relu_evac(o, acc):
            nc.scalar.activation(big[:, o, :], acc, AF.Relu)

        # ---------------- layers ----------------
        for l in range(NLAYERS):
            # ---- self-attention ----
            layernorm(xT)
            if l == 0:
                dump("dh1", hT, DK)
            proj(wqkv_sa[l], QKVO, DK, lambda o, k: hT[:, k, :], qkv_evac)
            if l == 0:
                dump("dqkv", big, QKVO)
            attention(masked=True)
            if l == 0:
                dump("dctx", ctxT, QO)
            # CA k/v depend only on the static memT: compute them during the
            # SA out-projection + AllReduce window to keep the PE busy.
            with tc.tile_pool(name="ov_ps", bufs=2, space="PSUM") as ovp:
                proj(wqkv_ca[l][4:QKVO], QKVO - 4, DK,
                     lambda o, k: memT[:, k, :],
                     lambda o, acc: qkv_evac(o + 4, acc),
                     pool=ovp, tag="kv")
                proj(wout_sa[l], DK, QO, lambda o, k: ctxT[:, k, :],
                     make_out_evac(), pool=ovp, tag="out")
            if l == 0:
                dump("dx1", xT, DK)

            # ---- cross-attention (k/v from packed embedding memT) ----
            layernorm(xT)
            if l == 0:
                dump("dh2", hT, DK)
            proj(wqkv_ca[l][0:4], 4, DK,
                 lambda o, k: hT[:, k, :], qkv_evac)
            if l == 0:
                dump("dqkv2", big, QKVO)
            attention(masked=False)
            if l == 0:
                dump("dctx2", ctxT, QO)
            proj(wout_ca[l], DK, QO, lambda o, k: ctxT[:, k, :],
                 make_out_evac())
            if l == 0:
                dump("dx2", xT, DK)

            # ---- FFN ----
            layernorm(xT)
            proj(w1_d[l], F1O, DK, lambda o, k: hT[:, k, :], relu_evac)
            if l == 0:
                dump("dff", big, F1O)
            proj(w2_d[l], DK, 2 * DK, lambda o, k: big[:, k, :],
                 make_out_evac())
            if l == 0:
                dump("dx3", xT, DK)

        # ---------------- head (vocab split across the pair) ----------------
        def head_evac(o, acc):
            for c in range(2):
                ev = prb.tile([P, 384], F32R, tag="p384")
                nc.scalar.copy(ev.bitcast(F32), acc[:, CH[c]:CH[c + 1]])
                nc.sync.dma_start(
                    out=logits[o * P:(o + 1) * P, CH[c]:CH[c + 1]],
                    in_=ev.bitcast(F32))

        proj(headw, HEADO, DK, lambda o, k: xT[:, k, :], head_evac)
        if DBG:
            nc.sync.dma_start(out=dbg["dvts"][:, :],
                              in_=vts[:, :, :, :].bitcast(F32R))
            nc.sync.dma_start(out=dbg["dinv"][:, :], in_=invb.bitcast(F32R))

    nc.finalize()
    return nc


# ---------------------------------------------------------------------------
# host side
# ---------------------------------------------------------------------------

def _pe_table(length, d):
    pos = np.arange(length, dtype=np.float32)[:, None]
    div = np.exp(np.arange(0, d, 2, dtype=np.float32) * (-np.log(10000.0) / d))
    ang = pos * div
    out = np.zeros((length, d), np.float32)
    out[:, 0::2] = np.sin(ang)
    out[:, 1::2] = np.cos(ang)
    return out


def _tp(w):
    """[..., O*128, K*128] -> [..., O, 128, K*128] pre-transposed tiles:
    out[..., o, p, k*128+c] = w[..., o*128+c, k*128+p]."""
    lead = w.shape[:-2]
    O, K = w.shape[-2] // P, w.shape[-1] // P
    w = w.reshape(*lead, O, P, K, P)
    axes = tuple(range(len(lead))) + tuple(
        len(lead) + a for a in (0, 3, 2, 1))
    return np.ascontiguousarray(
        w.transpose(*axes).reshape(*lead, O, P, K * P))


_NC_CACHE = {}
LAST_RESULT = {}


def kernel(**inputs):
    f32 = lambda a: np.ascontiguousarray(np.asarray(a, dtype=np.float32))
    text = np.asarray(inputs["text"]).astype(np.int64)
    audio = np.asarray(inputs["audio"]).astype(np.int64)
    tl = np.asarray(inputs["text_len_batch"]).astype(np.int64)
    al = np.asarray(inputs["audio_len_batch"]).astype(np.int64)
    text_table = f32(inputs["text_table"])
    audio_table = f32(inputs["audio_table"])
    sa_in_w = f32(inputs["sa_in_w"])
    sa_out_w = f32(inputs["sa_out_w"])
    ca_in_w = f32(inputs["ca_in_w"])
    ca_out_w = f32(inputs["ca_out_w"])
    ffn_w1 = f32(inputs["ffn_w1"])
    ffn_w2 = f32(inputs["ffn_w2"])

    comb = np.ascontiguousarray(np.concatenate(
        [text_table, audio_table, np.zeros((1, D), np.float32)], axis=0))
    pe_t = _pe_table(Tt, D)
    pe_a = _pe_table(Ta, D)

    in_maps = []
    for c in range(8):
        p, r = c // 2, c % 2
        tlb, alb = int(tl[p]), int(al[p])
        il = tlb + alb

        ids = np.full((L,), VT + VA, dtype=np.int64)  # default: zero row
        ids[:tlb] = text[p, :tlb]
        ids[tlb:il] = VT + audio[p, :alb]
        ids16 = np.ascontiguousarray(np.tile(ids.astype(np.int16).reshape(L // 16, 16).T, (8, 1)))

        pe_pack = np.zeros((L, D), np.float32)
        pe_pack[:tlb] = pe_t[:tlb]
        pe_pack[tlb:il] = pe_a[:alb]
        peT = np.ascontiguousarray(pe_pack.T)

        kk = np.arange(L)
        lo = np.where(kk < tlb, 0, kk).astype(np.float32)
        hi = np.where(kk < tlb, L, il).astype(np.float32)
        mlo = np.ascontiguousarray(lo.reshape(NT, P).T)          # [128, 6]
        mhi = np.ascontiguousarray((1.0 - hi).reshape(NT, P).T)

        sl = slice(512 * r, 512 * (r + 1))

        def qkv_shard(w3):
            qq = w3[:, 0:1024, :][:, sl]
            kx = w3[:, 1024:2048, :][:, sl]
            vv = w3[:, 2048:3072, :][:, sl]
            return np.ascontiguousarray(np.concatenate([qq, kx, vv], axis=1))

        hw = np.zeros((HEADO * P, D), np.float32)
        hw[0:513] = audio_table[513 * r:513 * (r + 1)]

        in_maps.append({
            "comb": comb, "ids16": ids16, "peT": peT,
            "mlo": mlo, "mhi": mhi,
            "wqkv_sa": _tp(qkv_shard(sa_in_w[:NLAYERS])),
            "wout_sa": _tp(np.ascontiguousarray(sa_out_w[:NLAYERS, :, sl])),
            "wqkv_ca": _tp(qkv_shard(ca_in_w[:NLAYERS])),
            "wout_ca": _tp(np.ascontiguousarray(ca_out_w[:NLAYERS, :, sl])),
            "w1": _tp(np.ascontiguousarray(
                ffn_w1[:NLAYERS, 2048 * r:2048 * (r + 1), :])),
            "w2": _tp(np.ascontiguousarray(
                ffn_w2[:NLAYERS, :, 2048 * r:2048 * (r + 1)])),
            "headw": _tp(hw),
        })

    if "nc" not in _NC_CACHE:
        _NC_CACHE["nc"] = _build_nc()
    nc = _NC_CACHE["nc"]
    trace = bool(int(os.environ.get("KERNEL_TRACE", "0")))
    r = run_bass_kernel_spmd(nc, in_maps, core_ids=list(range(8)), trace=trace)
    LAST_RESULT["r"] = r
    res = r.results

    out = np.empty((B, L, VA), np.float32)
    for p in range(B):
        ev = res[2 * p]["logits"]
        od = res[2 * p + 1]["logits"]
        out[p] = np.concatenate([ev[0:513], od[0:513]], axis=0).T
    return out
